# revision 7
# baseline (speedup 1.0000x reference)
"""Trainium2 Bass kernel for nn_CircumpunctAttention.

Full inputs in, full output out. Internally: data-parallel over batch (2) x
tensor-parallel over heads (4 head-groups of 4 heads) = 8 NeuronCores.

Per core: multi-head attention on 4 heads (= 2 pairs).  Head pair m is
stacked on the partition axis (head 2m at partitions 0-63, head 2m+1 at
64-127), so the two K=64 scores matmuls of a pair run CONCURRENTLY in the
PE array as row-tiles (tile_position auto-derived from base_partition) and
one exp activation per (pair, query-block, key-tile) covers both heads at
free dim 1024.  The ScalarE exp stream (16.8M exps at 1 elem/lane/cycle)
is the throughput wall; the scalar queue carries nothing but exp, and all
other work (projections, attnV, emerge, normalize) is scheduled into the
PE/DVE slack under it.

attnV uses the ones-column trick (lhsT = [v | 1], M=65) so the softmax
denominator accumulates in PSUM partition 64 alongside the weighted sum.
The reciprocal of the denominator is broadcast across partitions with a
tiny K=1 PE matmul against a constant ones row (no DRAM round-trip).

PSUM budget (8 banks): scores 2 bufs x [128,1024]f32 (4) + two [65,512]
attnV accumulators (2) + a [128,512] pool shared by v-proj / projection
quarters / emerge / normalize-broadcast (2).  Background projections are
emitted as 1-bank "quarters" so they never stall the scores double-buffer.

The per-head "aperture chamber" (valves, rotation, tanh gate) is folded
into We host-side in float64.  Softmax max-subtraction skipped: scores are
bounded (|s| < ~7).  All matmul operands fp16.
"""

import math
from contextlib import ExitStack
import numpy as np

# ---------------------------------------------------------------- constants
P = 128          # partitions
T = 2048         # sequence length
D = 1024         # model dim
H = 16           # total heads
DH = 64          # head dim
HC = 4           # heads per core
C = HC * DH      # channels per core (256)
KT = D // P      # 8 contraction tiles over model dim
TT = T // P      # 16 tiles over sequence
MT = C // P      # 2 partition tiles (= head pairs) per core
NJ = 4           # query blocks
JW = T // NJ     # query block width (512)
NCORES = 8
SCALE = 8.0      # sqrt(dh * conv_factor), conv_factor = 1/phi^0 = 1

CFG = {
    "dt": "float16",   # matmul operand dtype (storage); accum stays f32
}

LAST_EXEC_NS = None
_CACHE = {}


def _np_dt(name):
    if name == "bfloat16":
        import ml_dtypes
        return np.dtype(ml_dtypes.bfloat16)
    return np.dtype(name)


def build_nc(cfg=CFG):
    """Build + compile the single-core SPMD program."""
    import concourse.bass as bass
    import concourse.mybir as mybir
    import concourse.tile as tile
    from concourse import bacc

    dt = mybir.dt
    f32 = dt.float32
    dtx = getattr(dt, cfg["dt"])

    nc = bacc.Bacc("TRN2", target_bir_lowering=False, debug=False,
                   enable_asserts=False)

    xT = nc.dram_tensor("xt", [D, T], dtx, kind="ExternalInput").ap()
    wq = nc.dram_tensor("wq", [D, C], dtx, kind="ExternalInput").ap()
    wk = nc.dram_tensor("wk", [D, C], dtx, kind="ExternalInput").ap()
    wv = nc.dram_tensor("wv", [D, C], dtx, kind="ExternalInput").ap()
    we = nc.dram_tensor("we", [C, D], dtx, kind="ExternalInput").ap()
    out = nc.dram_tensor("out", [T, D], f32, kind="ExternalOutput").ap()

    Exp = mybir.ActivationFunctionType.Exp

    with tile.TileContext(nc) as tc, ExitStack() as ctx:
        cp = ctx.enter_context(tc.tile_pool(name="const", bufs=1))
        psp = ctx.enter_context(tc.tile_pool(name="psum", bufs=2,
                                             space="PSUM"))
        pso = ctx.enter_context(tc.tile_pool(name="psum_o", bufs=1,
                                             space="PSUM"))
        pse = ctx.enter_context(tc.tile_pool(name="psum_ve", bufs=2,
                                             space="PSUM"))
        p_pool = ctx.enter_context(tc.tile_pool(name="pp", bufs=4))
        u_pool = ctx.enter_context(tc.tile_pool(name="usb", bufs=2))
        nrm_b = ctx.enter_context(tc.tile_pool(name="nrm_b", bufs=2))
        out_pool = ctx.enter_context(tc.tile_pool(name="oute", bufs=2))

        xT_sb = cp.tile([P, KT, T], dtx)
        wq_sb = cp.tile([P, KT, C], dtx)
        wk_sb = cp.tile([P, KT, C], dtx)
        wv_sb = cp.tile([P, KT, C], dtx)
        we_sb = cp.tile([P, MT, D], dtx)
        qT_sb = cp.tile([P, MT, T], dtx)
        kT_sb = cp.tile([P, MT, T], dtx)
        # v augmented with a ones column at index DH (softmax denominator);
        # padded to 68 for 4-byte-aligned strides. cols 65-67 never read.
        va_sb = cp.tile([P, TT, HC, 68], dtx)
        oT_sb = cp.tile([P, MT, T], dtx)
        ones_r = cp.tile([1, DH], f32)       # lhsT of the bcast matmul
        dummy = cp.tile([1, 16], f32)
        nc.vector.memset(va_sb[:, :, :, DH:DH + 1], 1.0)
        nc.vector.memset(ones_r, 1.0)
        nc.vector.memset(dummy, 0.0)
        # warm the exp table while input DMAs run
        nc.scalar.activation(dummy, dummy, Exp)

        # ---- loads, spread over 4 queues (sync/gpsimd/vector/tensor).
        nc.sync.dma_start(out=wk_sb, in_=wk.rearrange("(k p) c -> p k c", p=P))
        nc.gpsimd.dma_start(out=wv_sb,
                            in_=wv.rearrange("(k p) c -> p k c", p=P))
        nc.scalar.dma_start(out=wq_sb,
                            in_=wq.rearrange("(k p) c -> p k c", p=P))
        engs = (nc.sync, nc.gpsimd, nc.scalar, nc.sync)
        for ch in range(4):
            kk = KT // 4
            engs[ch].dma_start(
                out=xT_sb[:, ch * kk:(ch + 1) * kk, :],
                in_=xT[ch * kk * P:(ch + 1) * kk * P, :].rearrange(
                    "(k p) t -> p k t", p=P))
        nc.gpsimd.dma_start(out=we_sb,
                            in_=we.rearrange("(m p) d -> p m d", p=P))

        def proj_full(w_sb, dst, m, jh):
            """Full [128,1024] projection in the scores pool (lead only)."""
            ps = psp.tile([P, T // 2], f32, tag="s")
            for k in range(KT):
                for c0 in range(0, T // 2, 512):
                    nc.tensor.matmul(
                        ps[:, c0:c0 + 512],
                        lhsT=w_sb[:, k, m * P:(m + 1) * P],
                        rhs=xT_sb[:, k, jh * (T // 2) + c0:
                                  jh * (T // 2) + c0 + 512],
                        start=(k == 0), stop=(k == KT - 1),
                    )
            nc.vector.tensor_copy(
                dst[:, m, jh * (T // 2):(jh + 1) * (T // 2)], ps)

        def proj_quarter(w_sb, dst, m, qq):
            """512-wide projection in the 1-bank ve pool: never blocks the
            scores double-buffer rotation."""
            ps = pse.tile([P, 512], f32, tag="ve")
            for k in range(KT):
                nc.tensor.matmul(
                    ps,
                    lhsT=w_sb[:, k, m * P:(m + 1) * P],
                    rhs=xT_sb[:, k, qq * 512:(qq + 1) * 512],
                    start=(k == 0), stop=(k == KT - 1),
                )
            nc.vector.tensor_copy(dst[:, m, qq * 512:(qq + 1) * 512], ps)

        def proj_v_tile(t):
            ps = pse.tile([P, C], f32, tag="ve")
            for k in range(KT):
                nc.tensor.matmul(
                    ps,
                    lhsT=xT_sb[:, k, t * P:(t + 1) * P],
                    rhs=wv_sb[:, k, :],
                    start=(k == 0), stop=(k == KT - 1),
                )
            nc.vector.tensor_copy(
                va_sb[:, t, :, 0:DH],
                ps.rearrange("p (h d) -> p h d", h=HC))

        def normalize(m, jq, hh, tail=False):
            """po -> oT with 1/denominator; PE-matmul broadcast of the
            reciprocal row (no DRAM bounce)."""
            q0 = jq * JW
            po = _po[hh]
            uv = u_pool.tile([DH, JW], f32, tag="u")
            nc.vector.tensor_copy(uv, po[0:DH, :])
            ud = u_pool.tile([1, JW], f32, tag="ud")
            (nc.scalar.copy if tail else nc.vector.tensor_copy)(
                ud, po[DH:DH + 1, :])
            rr = nrm_b.tile([1, JW], f32, tag="rr")
            nc.vector.reciprocal_approx_fast(rr, ud)
            rb = pse.tile([DH, JW], f32, tag="ve")
            nc.tensor.matmul(rb, lhsT=ones_r, rhs=rr, start=True, stop=True)
            if hh == 0:
                nc.vector.tensor_mul(
                    oT_sb[0:DH, m, q0:q0 + JW], uv, rb)
            else:
                st = nrm_b.tile([DH, JW], dtx, tag="st")
                nc.vector.tensor_mul(st, uv, rb)
                eng = nc.sync if m == 0 else nc.gpsimd
                eng.dma_start(out=oT_sb[DH:P, m, q0:q0 + JW], in_=st)

        _po = [None, None]

        def attention(m, jq, fills=(), pre_kt=None):
            """Head pair m, query block jq: scores (row-tiled pair), exp
            (one call for both heads), attnV with ones-column denominator.
            fills: {kt: emitter} of background PE work slotted after the
            scores of that kt."""
            q0 = jq * JW
            for hh in range(2):
                _po[hh] = pso.tile([DH + 1, JW], f32, tag=f"o{hh}",
                                   name=f"po{hh}")
            for kt in range(TT):
                ps = psp.tile([P, 2 * JW], f32, tag="s")
                for hh in range(2):
                    nc.tensor.matmul(
                        ps[:, hh * JW:(hh + 1) * JW],
                        lhsT=kT_sb[hh * DH:(hh + 1) * DH, m,
                                   kt * P:(kt + 1) * P],
                        rhs=qT_sb[hh * DH:(hh + 1) * DH, m, q0:q0 + JW],
                        start=True, stop=True,
                    )
                if kt in fills:
                    fills[kt]()
                if pre_kt is not None:
                    pre_kt(kt)
                p_t = p_pool.tile([P, 2 * JW], dtx, tag="p")
                nc.scalar.activation(p_t, ps, Exp)
                for hh in range(2):
                    nc.tensor.matmul(
                        _po[hh],
                        lhsT=va_sb[:, kt, 2 * m + hh, 0:DH + 1],
                        rhs=p_t[:, hh * JW:(hh + 1) * JW],
                        start=(kt == 0), stop=(kt == TT - 1),
                    )
            tail = (m == 1 and jq == NJ - 1)
            normalize(m, jq, 0, tail)
            normalize(m, jq, 1, tail)

        def emerge_t(t, tail=False):
            """out[t,:]: one query tile through the folded We."""
            ob = out_pool.tile([P, D], f32, tag="ob")
            for ci, c0 in enumerate(range(0, D, 512)):
                pe = pse.tile([P, 512], f32, tag="ve")
                for mm in range(MT):
                    nc.tensor.matmul(
                        pe,
                        lhsT=oT_sb[:, mm, t * P:(t + 1) * P],
                        rhs=we_sb[:, mm, c0:c0 + 512],
                        start=(mm == 0), stop=(mm == MT - 1),
                    )
                if tail and ci == 1:
                    nc.scalar.copy(ob[:, c0:c0 + 512], pe)
                else:
                    nc.vector.tensor_copy(ob[:, c0:c0 + 512], pe)
            eng = nc.sync if t % 2 == 0 else nc.gpsimd
            eng.dma_start(out=out[t * P:(t + 1) * P, :], in_=ob)

        # ---- emission = per-engine queue order.  All m=0 blocks first so
        # pair-1 projections hide in m=0's ACT-bound slack; emerges hide in
        # the m=1 blocks; the v projection rides the first block.
        proj_full(wk_sb, kT_sb, 0, 0)
        proj_full(wq_sb, qT_sb, 0, 0)
        attention(0, 0, fills={
            1: lambda: proj_quarter(wk_sb, kT_sb, 0, 2),
            5: lambda: proj_quarter(wk_sb, kT_sb, 0, 3),
        }, pre_kt=proj_v_tile)
        attention(0, 1, fills={
            0: lambda: proj_quarter(wq_sb, qT_sb, 0, 2),
            2: lambda: proj_quarter(wq_sb, qT_sb, 0, 3),
            4: lambda: proj_quarter(wk_sb, kT_sb, 1, 0),
            6: lambda: proj_quarter(wk_sb, kT_sb, 1, 1),
            8: lambda: proj_quarter(wk_sb, kT_sb, 1, 2),
            10: lambda: proj_quarter(wk_sb, kT_sb, 1, 3),
            12: lambda: proj_quarter(wq_sb, qT_sb, 1, 0),
            14: lambda: proj_quarter(wq_sb, qT_sb, 1, 1),
        })
        attention(0, 2, fills={
            0: lambda: proj_quarter(wq_sb, qT_sb, 1, 2),
            2: lambda: proj_quarter(wq_sb, qT_sb, 1, 3),
        })
        attention(0, 3)
        attention(1, 0)
        attention(1, 1, fills={
            0: lambda: emerge_t(0), 4: lambda: emerge_t(1),
            8: lambda: emerge_t(2), 12: lambda: emerge_t(3),
        })
        attention(1, 2, fills={
            0: lambda: emerge_t(4), 4: lambda: emerge_t(5),
            8: lambda: emerge_t(6), 12: lambda: emerge_t(7),
        })
        attention(1, 3, fills={
            0: lambda: emerge_t(8), 4: lambda: emerge_t(9),
            8: lambda: emerge_t(10), 12: lambda: emerge_t(11),
        })
        for t in range(12, 16):
            emerge_t(t, tail=True)

    nc.compile()
    return nc


def prep_inputs(x, Wq, Wk, Wv, We, beta, input_valve, output_valve, chi,
                cfg=CFG):
    """Host-side prep: fold chamber into We, fold 1/scale into Wq, shard."""
    x = np.asarray(x, np.float32)
    Wq = np.asarray(Wq, np.float32)
    Wk = np.asarray(Wk, np.float32)
    Wv = np.asarray(Wv, np.float32)
    We = np.asarray(We, np.float32)

    def sig(v):
        return 1.0 / (1.0 + np.exp(-np.asarray(v, np.float64)))

    b = sig(beta)
    iv = sig(input_valve)
    ov = sig(output_valve)
    g = np.tanh(np.asarray(chi, np.float64))
    ang = math.pi * b
    ca, sa = np.cos(ang), np.sin(ang)
    half = DH // 2

    We64 = We.astype(np.float64)
    WeP = np.empty((D, D), np.float64)
    for h in range(H):
        L = np.zeros((DH, DH))
        idx = np.arange(half)
        L[idx, idx] = ca[h]
        L[idx, half + idx] = -sa[h]
        L[half + idx, idx] = sa[h]
        L[half + idx, half + idx] = ca[h]
        L *= ov[h] * g[h] * iv[h]
        WeP[:, h * DH:(h + 1) * DH] = We64[:, h * DH:(h + 1) * DH] @ L

    dt_x = _np_dt(cfg["dt"])
    WqT = np.ascontiguousarray((Wq.astype(np.float64) / SCALE).T, dt_x)
    WkT = np.ascontiguousarray(Wk.T, dt_x)
    WvT = np.ascontiguousarray(Wv.T, dt_x)
    WeT = np.ascontiguousarray(WeP.T, dt_x)   # [c, dout]

    in_maps = []
    for core in range(NCORES):
        bidx, grp = divmod(core, H // HC)
        cols = slice(grp * C, (grp + 1) * C)
        in_maps.append({
            "xt": np.ascontiguousarray(x[bidx].T.astype(dt_x)),
            "wq": np.ascontiguousarray(WqT[:, cols]),
            "wk": np.ascontiguousarray(WkT[:, cols]),
            "wv": np.ascontiguousarray(WvT[:, cols]),
            "we": np.ascontiguousarray(WeT[cols, :]),
        })
    return in_maps


def kernel(**inputs):
    global LAST_EXEC_NS
    import os
    if "nc" not in _CACHE:
        _CACHE["nc"] = build_nc()
    nc = _CACHE["nc"]
    in_maps = prep_inputs(**inputs)

    from concourse.bass_utils import run_bass_kernel_spmd
    trace = bool(os.environ.get("CIRC_TRACE"))
    res = run_bass_kernel_spmd(nc, in_maps, list(range(NCORES)), trace=trace)
    LAST_EXEC_NS = res.exec_time_ns
    _CACHE["last_results"] = res

    B = 2
    outp = np.zeros((B, T, D), np.float32)
    per_batch = NCORES // B
    for core in range(NCORES):
        outp[core // per_batch] += res.results[core]["out"]
    return outp


# revision 12
# speedup vs baseline: 1.1028x; 1.1028x over previous
"""Trainium2 Bass kernel for nn_CircumpunctAttention.

Full inputs in, full output out. Internally: data-parallel over batch (2) x
tensor-parallel over heads (4 head-groups of 4 heads) = 8 NeuronCores.

Per core: multi-head attention on 4 heads (= 2 pairs).  Head pair m is
stacked on the partition axis (head 2m at partitions 0-63, head 2m+1 at
64-127), so the two K=64 scores matmuls of a pair run CONCURRENTLY in the
PE array as row-tiles (tile_position auto-derived from base_partition) and
one exp activation per (pair, query-block, key-tile) covers both heads at
free dim 1024.  The ScalarE exp stream (16.8M exps at 1 elem/lane/cycle)
is the throughput wall; the scalar queue carries nothing but exp, and all
other work (projections, attnV, emerge, normalize) is scheduled into the
PE/DVE slack under it.

attnV uses the ones-column trick (lhsT = [v | 1], M=65) so the softmax
denominator accumulates in PSUM partition 64 alongside the weighted sum.
The reciprocal of the denominator is broadcast across partitions with a
tiny K=1 PE matmul against a constant ones row (no DRAM round-trip).

PSUM budget (8 banks): scores 2 bufs x [128,1024]f32 (4) + two [65,512]
attnV accumulators (2) + a [128,512] pool shared by v-proj / projection
quarters / emerge / normalize-broadcast (2).  Background projections are
emitted as 1-bank "quarters" so they never stall the scores double-buffer.

The per-head "aperture chamber" (valves, rotation, tanh gate) is folded
into We host-side in float64.  Softmax max-subtraction skipped: scores are
bounded (|s| < ~7).  All matmul operands fp16.
"""

import math
from contextlib import ExitStack
import numpy as np

# ---------------------------------------------------------------- constants
P = 128          # partitions
T = 2048         # sequence length
D = 1024         # model dim
H = 16           # total heads
DH = 64          # head dim
HC = 4           # heads per core
C = HC * DH      # channels per core (256)
KT = D // P      # 8 contraction tiles over model dim
TT = T // P      # 16 tiles over sequence
MT = C // P      # 2 partition tiles (= head pairs) per core
NJ = 4           # query blocks
JW = T // NJ     # query block width (512)
NCORES = 8
SCALE = 8.0      # sqrt(dh * conv_factor), conv_factor = 1/phi^0 = 1

CFG = {
    "dt": "float16",   # matmul operand dtype (storage); accum stays f32
}

LAST_EXEC_NS = None
_CACHE = {}


def _np_dt(name):
    if name == "bfloat16":
        import ml_dtypes
        return np.dtype(ml_dtypes.bfloat16)
    return np.dtype(name)


def build_nc(cfg=CFG):
    """Build + compile the single-core SPMD program."""
    import concourse.bass as bass
    import concourse.mybir as mybir
    import concourse.tile as tile
    from concourse import bacc

    dt = mybir.dt
    f32 = dt.float32
    dtx = getattr(dt, cfg["dt"])

    nc = bacc.Bacc("TRN2", target_bir_lowering=False, debug=False,
                   enable_asserts=False)

    xT = nc.dram_tensor("xt", [D, T], dtx, kind="ExternalInput").ap()
    wq = nc.dram_tensor("wq", [D, C], dtx, kind="ExternalInput").ap()
    wk = nc.dram_tensor("wk", [D, C], dtx, kind="ExternalInput").ap()
    wv = nc.dram_tensor("wv", [D, C], dtx, kind="ExternalInput").ap()
    we = nc.dram_tensor("we", [C, D], dtx, kind="ExternalInput").ap()
    out = nc.dram_tensor("out", [T, D], f32, kind="ExternalOutput").ap()

    Exp = mybir.ActivationFunctionType.Exp

    with tile.TileContext(nc) as tc, ExitStack() as ctx:
        cp = ctx.enter_context(tc.tile_pool(name="const", bufs=1))
        psp = ctx.enter_context(tc.tile_pool(name="psum", bufs=2,
                                             space="PSUM"))
        pso = ctx.enter_context(tc.tile_pool(name="psum_o", bufs=1,
                                             space="PSUM"))
        pse = ctx.enter_context(tc.tile_pool(name="psum_ve", bufs=2,
                                             space="PSUM"))
        p_pool = ctx.enter_context(tc.tile_pool(name="pp", bufs=4))
        u_pool = ctx.enter_context(tc.tile_pool(name="usb", bufs=2))
        nrm_b = ctx.enter_context(tc.tile_pool(name="nrm_b", bufs=2))
        nrm_d = ctx.enter_context(tc.tile_pool(name="nrm_d", bufs=2,
                                               space="DRAM"))
        out_pool = ctx.enter_context(tc.tile_pool(name="oute", bufs=2))

        xT_sb = cp.tile([P, KT, T], dtx)
        wq_sb = cp.tile([P, KT, C], dtx)
        wk_sb = cp.tile([P, KT, C], dtx)
        wv_sb = cp.tile([P, KT, C], dtx)
        we_sb = cp.tile([P, MT, D], dtx)
        qT_sb = cp.tile([P, MT, T], dtx)
        kT_sb = cp.tile([P, MT, T], dtx)
        # v augmented with a ones column at index DH (softmax denominator);
        # padded to 68 for 4-byte-aligned strides. cols 65-67 never read.
        va_sb = cp.tile([P, TT, HC, 68], dtx)
        oT_sb = cp.tile([P, MT, T], dtx)
        ones_r = cp.tile([1, DH], f32)       # lhsT of the bcast matmul
        dummy = cp.tile([1, 16], f32)
        nc.vector.memset(va_sb[:, :, :, DH:DH + 1], 1.0)
        nc.vector.memset(ones_r, 1.0)
        nc.vector.memset(dummy, 0.0)
        # warm the exp table while input DMAs run
        nc.scalar.activation(dummy, dummy, Exp)

        # ---- loads.  DMA engines fair-share bandwidth across in-flight
        # transfers, so the critical-path loads (wk/wq + xT chunks) are
        # issued strictly first and everything else behind them.
        nc.sync.dma_start(out=wk_sb, in_=wk.rearrange("(k p) c -> p k c", p=P))
        nc.gpsimd.dma_start(out=wq_sb,
                            in_=wq.rearrange("(k p) c -> p k c", p=P))
        for ch in range(4):
            kk = KT // 4
            (nc.sync, nc.gpsimd)[ch % 2].dma_start(
                out=xT_sb[:, ch * kk:(ch + 1) * kk, :],
                in_=xT[ch * kk * P:(ch + 1) * kk * P, :].rearrange(
                    "(k p) t -> p k t", p=P))
        nc.gpsimd.dma_start(out=wv_sb,
                            in_=wv.rearrange("(k p) c -> p k c", p=P))
        nc.sync.dma_start(out=we_sb,
                            in_=we.rearrange("(m p) d -> p m d", p=P))

        def proj_kq_lead():
            """k and q projections for pair 0's first query/key half,
            interleaved per k-tile so both finish right after the xT DMA."""
            psk = psp.tile([P, T // 2], f32, tag="s", name="psk")
            psq = psp.tile([P, T // 2], f32, tag="s", name="psq")
            for k in range(KT):
                for ps, w_sb in ((psk, wk_sb), (psq, wq_sb)):
                    for c0 in range(0, T // 2, 512):
                        nc.tensor.matmul(
                            ps[:, c0:c0 + 512],
                            lhsT=w_sb[:, k, 0:P],
                            rhs=xT_sb[:, k, c0:c0 + 512],
                            start=(k == 0), stop=(k == KT - 1),
                        )
            nc.vector.tensor_copy(kT_sb[:, 0, 0:T // 2], psk)
            nc.vector.tensor_copy(qT_sb[:, 0, 0:T // 2], psq)

        def proj_quarter(w_sb, dst, m, qq):
            """512-wide projection in the 1-bank ve pool: never blocks the
            scores double-buffer rotation."""
            ps = pse.tile([P, 512], f32, tag="ve")
            for k in range(KT):
                nc.tensor.matmul(
                    ps,
                    lhsT=w_sb[:, k, m * P:(m + 1) * P],
                    rhs=xT_sb[:, k, qq * 512:(qq + 1) * 512],
                    start=(k == 0), stop=(k == KT - 1),
                )
            nc.vector.tensor_copy(dst[:, m, qq * 512:(qq + 1) * 512], ps)

        def proj_v_tile(t):
            ps = pse.tile([P, C], f32, tag="ve")
            for k in range(KT):
                nc.tensor.matmul(
                    ps,
                    lhsT=xT_sb[:, k, t * P:(t + 1) * P],
                    rhs=wv_sb[:, k, :],
                    start=(k == 0), stop=(k == KT - 1),
                )
            nc.vector.tensor_copy(
                va_sb[:, t, :, 0:DH],
                ps.rearrange("p (h d) -> p h d", h=HC))

        def normalize_bounce(prev):
            """DMA-bounce normalize of the PREVIOUS block, emitted inside
            the next block so the DRAM round-trip latency is hidden.  The
            u-copy frees the po banks within ~1us of the block start."""
            m, jq, pos = prev
            q0 = jq * JW
            for hh in range(2):
                u = u_pool.tile([DH + 1, JW], f32, tag="u")
                nc.vector.tensor_copy(u, pos[hh])
                r_dr = nrm_d.tile([1, JW], f32, tag="rd")
                eng = nc.sync if hh == 0 else nc.gpsimd
                eng.dma_start(out=r_dr, in_=u[DH:DH + 1, :])
                lbc = nrm_b.tile([DH, JW], f32, tag="lbc")
                eng.dma_start(out=lbc, in_=r_dr.to_broadcast((DH, JW)))
                rbc = nrm_b.tile([DH, JW], f32, tag="rbc")
                nc.vector.reciprocal_approx_fast(rbc, lbc)
                if hh == 0:
                    nc.vector.tensor_mul(
                        oT_sb[0:DH, m, q0:q0 + JW], u[0:DH, :], rbc)
                else:
                    st = nrm_b.tile([DH, JW], dtx, tag="st")
                    nc.vector.tensor_mul(st, u[0:DH, :], rbc)
                    eng.dma_start(out=oT_sb[DH:P, m, q0:q0 + JW], in_=st)

        def normalize_tail(prev):
            """Fast-path normalize for the last block: reciprocal row
            broadcast via a K=1 PE matmul, scalar engine assists."""
            m, jq, pos = prev
            q0 = jq * JW
            for hh in range(2):
                uv = u_pool.tile([DH, JW], f32, tag="u2")
                ud = u_pool.tile([1, JW], f32, tag="ud")
                (nc.vector.tensor_copy if hh == 0 else nc.scalar.copy)(
                    uv, pos[hh][0:DH, :])
                (nc.scalar.copy if hh == 0 else nc.vector.tensor_copy)(
                    ud, pos[hh][DH:DH + 1, :])
                rr = nrm_b.tile([1, JW], f32, tag="rr")
                nc.vector.reciprocal_approx_fast(rr, ud)
                rb = pse.tile([DH, JW], f32, tag="ve")
                nc.tensor.matmul(rb, lhsT=ones_r, rhs=rr,
                                 start=True, stop=True)
                if hh == 0:
                    nc.vector.tensor_mul(
                        oT_sb[0:DH, m, q0:q0 + JW], uv, rb)
                else:
                    st = nrm_b.tile([DH, JW], dtx, tag="st")
                    nc.vector.tensor_mul(st, uv, rb)
                    nc.gpsimd.dma_start(
                        out=oT_sb[DH:P, m, q0:q0 + JW], in_=st)

        def attention(m, jq, fills=(), pre_kt=None, norm_prev=None):
            """Head pair m, query block jq: scores (row-tiled pair), exp
            (one call for both heads), attnV with ones-column denominator.
            fills: {kt: emitter} of background PE work slotted after the
            scores of that kt.  Returns (m, jq, po-tiles) for deferred
            normalization inside the next block."""
            q0 = jq * JW
            pos = [pso.tile([DH + 1, JW], f32, tag=f"o{hh}", name=f"po{hh}")
                   for hh in range(2)]
            for kt in range(TT):
                ps = psp.tile([P, 2 * JW], f32, tag="s")
                for hh in range(2):
                    nc.tensor.matmul(
                        ps[:, hh * JW:(hh + 1) * JW],
                        lhsT=kT_sb[hh * DH:(hh + 1) * DH, m,
                                   kt * P:(kt + 1) * P],
                        rhs=qT_sb[hh * DH:(hh + 1) * DH, m, q0:q0 + JW],
                        start=True, stop=True,
                    )
                if kt == 0 and norm_prev is not None:
                    normalize_bounce(norm_prev)
                if kt in fills:
                    fills[kt]()
                if pre_kt is not None:
                    pre_kt(kt)
                p_t = p_pool.tile([P, 2 * JW], dtx, tag="p")
                nc.scalar.activation(p_t, ps, Exp)
                for hh in range(2):
                    nc.tensor.matmul(
                        pos[hh],
                        lhsT=va_sb[:, kt, 2 * m + hh, 0:DH + 1],
                        rhs=p_t[:, hh * JW:(hh + 1) * JW],
                        start=(kt == 0), stop=(kt == TT - 1),
                    )
            return (m, jq, pos)

        def emerge_t(t, tail=False):
            """out[t,:]: one query tile through the folded We."""
            ob = out_pool.tile([P, D], f32, tag="ob")
            for ci, c0 in enumerate(range(0, D, 512)):
                pe = pse.tile([P, 512], f32, tag="ve")
                for mm in range(MT):
                    nc.tensor.matmul(
                        pe,
                        lhsT=oT_sb[:, mm, t * P:(t + 1) * P],
                        rhs=we_sb[:, mm, c0:c0 + 512],
                        start=(mm == 0), stop=(mm == MT - 1),
                    )
                if tail and ci == 1:
                    nc.scalar.copy(ob[:, c0:c0 + 512], pe)
                else:
                    nc.vector.tensor_copy(ob[:, c0:c0 + 512], pe)
            eng = nc.sync if t % 2 == 0 else nc.gpsimd
            eng.dma_start(out=out[t * P:(t + 1) * P, :], in_=ob)

        # ---- emission = per-engine queue order.  All m=0 blocks first so
        # pair-1 projections hide in m=0's slack; emerges hide in the m=1
        # blocks; the v projection rides the first block; each block's
        # normalize is a kt=0 fill of the NEXT block (bounce hidden).
        proj_kq_lead()
        pv = attention(0, 0, fills={
            2: lambda: proj_quarter(wk_sb, kT_sb, 0, 2),
            5: lambda: proj_quarter(wk_sb, kT_sb, 0, 3),
        }, pre_kt=proj_v_tile)
        pv = attention(0, 1, norm_prev=pv, fills={
            2: lambda: proj_quarter(wq_sb, qT_sb, 0, 2),
            4: lambda: proj_quarter(wq_sb, qT_sb, 0, 3),
            6: lambda: proj_quarter(wk_sb, kT_sb, 1, 0),
            8: lambda: proj_quarter(wk_sb, kT_sb, 1, 1),
            10: lambda: proj_quarter(wk_sb, kT_sb, 1, 2),
            12: lambda: proj_quarter(wk_sb, kT_sb, 1, 3),
        })
        pv = attention(0, 2, norm_prev=pv, fills={
            2: lambda: proj_quarter(wq_sb, qT_sb, 1, 0),
            5: lambda: proj_quarter(wq_sb, qT_sb, 1, 1),
            8: lambda: proj_quarter(wq_sb, qT_sb, 1, 2),
            11: lambda: proj_quarter(wq_sb, qT_sb, 1, 3),
        })
        pv = attention(0, 3, norm_prev=pv)
        pv = attention(1, 0, norm_prev=pv)
        pv = attention(1, 1, norm_prev=pv, fills={
            6: lambda: emerge_t(0), 9: lambda: emerge_t(1),
            12: lambda: emerge_t(2), 15: lambda: emerge_t(3),
        })
        pv = attention(1, 2, norm_prev=pv, fills={
            6: lambda: emerge_t(4), 8: lambda: emerge_t(5),
            10: lambda: emerge_t(6), 12: lambda: emerge_t(7),
        })
        pv = attention(1, 3, norm_prev=pv, fills={
            6: lambda: emerge_t(8), 8: lambda: emerge_t(9),
            10: lambda: emerge_t(10), 12: lambda: emerge_t(11),
        })
        normalize_tail(pv)
        for t in range(12, 16):
            emerge_t(t, tail=True)

    nc.compile()
    return nc


def prep_inputs(x, Wq, Wk, Wv, We, beta, input_valve, output_valve, chi,
                cfg=CFG):
    """Host-side prep: fold chamber into We, fold 1/scale into Wq, shard."""
    x = np.asarray(x, np.float32)
    Wq = np.asarray(Wq, np.float32)
    Wk = np.asarray(Wk, np.float32)
    Wv = np.asarray(Wv, np.float32)
    We = np.asarray(We, np.float32)

    def sig(v):
        return 1.0 / (1.0 + np.exp(-np.asarray(v, np.float64)))

    b = sig(beta)
    iv = sig(input_valve)
    ov = sig(output_valve)
    g = np.tanh(np.asarray(chi, np.float64))
    ang = math.pi * b
    ca, sa = np.cos(ang), np.sin(ang)
    half = DH // 2

    We64 = We.astype(np.float64)
    WeP = np.empty((D, D), np.float64)
    for h in range(H):
        L = np.zeros((DH, DH))
        idx = np.arange(half)
        L[idx, idx] = ca[h]
        L[idx, half + idx] = -sa[h]
        L[half + idx, idx] = sa[h]
        L[half + idx, half + idx] = ca[h]
        L *= ov[h] * g[h] * iv[h]
        WeP[:, h * DH:(h + 1) * DH] = We64[:, h * DH:(h + 1) * DH] @ L

    dt_x = _np_dt(cfg["dt"])
    WqT = np.ascontiguousarray((Wq.astype(np.float64) / SCALE).T, dt_x)
    WkT = np.ascontiguousarray(Wk.T, dt_x)
    WvT = np.ascontiguousarray(Wv.T, dt_x)
    WeT = np.ascontiguousarray(WeP.T, dt_x)   # [c, dout]

    in_maps = []
    for core in range(NCORES):
        bidx, grp = divmod(core, H // HC)
        cols = slice(grp * C, (grp + 1) * C)
        in_maps.append({
            "xt": np.ascontiguousarray(x[bidx].T.astype(dt_x)),
            "wq": np.ascontiguousarray(WqT[:, cols]),
            "wk": np.ascontiguousarray(WkT[:, cols]),
            "wv": np.ascontiguousarray(WvT[:, cols]),
            "we": np.ascontiguousarray(WeT[cols, :]),
        })
    return in_maps


def kernel(**inputs):
    global LAST_EXEC_NS
    import os
    if "nc" not in _CACHE:
        _CACHE["nc"] = build_nc()
    nc = _CACHE["nc"]
    in_maps = prep_inputs(**inputs)

    from concourse.bass_utils import run_bass_kernel_spmd
    trace = bool(os.environ.get("CIRC_TRACE"))
    res = run_bass_kernel_spmd(nc, in_maps, list(range(NCORES)), trace=trace)
    LAST_EXEC_NS = res.exec_time_ns
    _CACHE["last_results"] = res

    B = 2
    outp = np.zeros((B, T, D), np.float32)
    per_batch = NCORES // B
    for core in range(NCORES):
        outp[core // per_batch] += res.results[core]["out"]
    return outp


# revision 17
# speedup vs baseline: 1.1155x; 1.0115x over previous
"""Trainium2 Bass kernel for nn_CircumpunctAttention.

Full inputs in, full output out. Internally: data-parallel over batch (2) x
tensor-parallel over heads (4 head-groups of 4 heads) = 8 NeuronCores.

Per core: multi-head attention on 4 heads (= 2 pairs).  Head pair m is
stacked on the partition axis (head 2m at partitions 0-63, head 2m+1 at
64-127), so the two K=64 scores matmuls of a pair run CONCURRENTLY in the
PE array as row-tiles (tile_position auto-derived from base_partition) and
one exp activation per (pair, query-block, key-tile) covers both heads at
free dim 1024.  The ScalarE exp stream (16.8M exps at 1 elem/lane/cycle)
is the throughput wall; the scalar queue carries nothing but exp, and all
other work (projections, attnV, emerge, normalize) is scheduled into the
PE/DVE slack under it.

attnV uses the ones-column trick (lhsT = [v | 1], M=65) so the softmax
denominator accumulates in PSUM partition 64 alongside the weighted sum.
The reciprocal of the denominator is broadcast across partitions with a
tiny K=1 PE matmul against a constant ones row (no DRAM round-trip).

PSUM budget (8 banks): scores 2 bufs x [128,1024]f32 (4) + two [65,512]
attnV accumulators (2) + a [128,512] pool shared by v-proj / projection
quarters / emerge / normalize-broadcast (2).  Background projections are
emitted as 1-bank "quarters" so they never stall the scores double-buffer.

The per-head "aperture chamber" (valves, rotation, tanh gate) is folded
into We host-side in float64.  Softmax max-subtraction skipped: scores are
bounded (|s| < ~7).  All matmul operands fp16.
"""

import math
from contextlib import ExitStack
import numpy as np

# ---------------------------------------------------------------- constants
P = 128          # partitions
T = 2048         # sequence length
D = 1024         # model dim
H = 16           # total heads
DH = 64          # head dim
HC = 4           # heads per core
C = HC * DH      # channels per core (256)
KT = D // P      # 8 contraction tiles over model dim
TT = T // P      # 16 tiles over sequence
MT = C // P      # 2 partition tiles (= head pairs) per core
NJ = 4           # query blocks
JW = T // NJ     # query block width (512)
NCORES = 8
SCALE = 8.0      # sqrt(dh * conv_factor), conv_factor = 1/phi^0 = 1

CFG = {
    "dt": "float16",   # matmul operand dtype (storage); accum stays f32
}

LAST_EXEC_NS = None
_CACHE = {}


def _np_dt(name):
    if name == "bfloat16":
        import ml_dtypes
        return np.dtype(ml_dtypes.bfloat16)
    return np.dtype(name)


def build_nc(cfg=CFG):
    """Build + compile the single-core SPMD program."""
    import concourse.bass as bass
    import concourse.mybir as mybir
    import concourse.tile as tile
    from concourse import bacc

    dt = mybir.dt
    f32 = dt.float32
    dtx = getattr(dt, cfg["dt"])

    nc = bacc.Bacc("TRN2", target_bir_lowering=False, debug=False,
                   enable_asserts=False)

    xT = nc.dram_tensor("xt", [D, T], dtx, kind="ExternalInput").ap()
    wq = nc.dram_tensor("wq", [D, C], dtx, kind="ExternalInput").ap()
    wk = nc.dram_tensor("wk", [D, C], dtx, kind="ExternalInput").ap()
    wv = nc.dram_tensor("wv", [D, C], dtx, kind="ExternalInput").ap()
    we = nc.dram_tensor("we", [C, D], dtx, kind="ExternalInput").ap()
    out = nc.dram_tensor("out", [T, D], f32, kind="ExternalOutput").ap()

    Exp = mybir.ActivationFunctionType.Exp

    with tile.TileContext(nc) as tc, ExitStack() as ctx:
        cp = ctx.enter_context(tc.tile_pool(name="const", bufs=1))
        psp = ctx.enter_context(tc.tile_pool(name="psum", bufs=2,
                                             space="PSUM"))
        pso = ctx.enter_context(tc.tile_pool(name="psum_o", bufs=1,
                                             space="PSUM"))
        pse = ctx.enter_context(tc.tile_pool(name="psum_ve", bufs=2,
                                             space="PSUM"))
        p_pool = ctx.enter_context(tc.tile_pool(name="pp", bufs=4))
        u_pool = ctx.enter_context(tc.tile_pool(name="usb", bufs=2))
        nrm_b = ctx.enter_context(tc.tile_pool(name="nrm_b", bufs=2))
        nrm_d = ctx.enter_context(tc.tile_pool(name="nrm_d", bufs=2,
                                               space="DRAM"))
        out_pool = ctx.enter_context(tc.tile_pool(name="oute", bufs=2))

        xT_sb = cp.tile([P, KT, T], dtx)
        wq_sb = cp.tile([P, KT, C], dtx)
        wk_sb = cp.tile([P, KT, C], dtx)
        wv_sb = cp.tile([P, KT, C], dtx)
        we_sb = cp.tile([P, MT, D], dtx)
        qT_sb = cp.tile([P, MT, T], dtx)
        kT_sb = cp.tile([P, MT, T], dtx)
        # v augmented with a ones column at index DH (softmax denominator);
        # padded to 68 for 4-byte-aligned strides. cols 65-67 never read.
        va_sb = cp.tile([P, TT, HC, 68], dtx)
        oT_sb = cp.tile([P, MT, T], dtx)
        ones_r = cp.tile([1, DH], f32)       # lhsT of the bcast matmul
        dummy = cp.tile([1, 16], f32)
        nc.vector.memset(va_sb[:, :, :, DH:DH + 1], 1.0)
        nc.vector.memset(ones_r, 1.0)
        nc.vector.memset(dummy, 0.0)
        # warm the exp table while input DMAs run
        nc.scalar.activation(dummy, dummy, Exp)

        # ---- loads.  DMA engines fair-share bandwidth across in-flight
        # transfers, so the critical-path loads (wk/wq + xT chunks) are
        # issued strictly first; xT is chunked 6 ways across the three
        # DMA-capable queues (sync, gpsimd, scalar).
        nc.sync.dma_start(out=wk_sb, in_=wk.rearrange("(k p) c -> p k c", p=P))
        nc.gpsimd.dma_start(out=wq_sb,
                            in_=wq.rearrange("(k p) c -> p k c", p=P))
        nc.scalar.dma_start(out=wv_sb,
                            in_=wv.rearrange("(k p) c -> p k c", p=P))
        dq = (nc.sync, nc.gpsimd, nc.scalar)
        xbounds = [0, 1, 2, 3, 4, 6, 8]
        for ch in range(6):
            a, b = xbounds[ch], xbounds[ch + 1]
            dq[ch % 3].dma_start(
                out=xT_sb[:, a:b, :],
                in_=xT[a * P:b * P, :].rearrange("(k p) t -> p k t", p=P))
        nc.sync.dma_start(out=we_sb,
                            in_=we.rearrange("(m p) d -> p m d", p=P))

        def proj_kq_lead():
            """k and q projections for pair 0's first query/key half,
            interleaved per k-tile so both finish right after the xT DMA;
            casts split in halves to release the first scores earlier."""
            psk = psp.tile([P, T // 2], f32, tag="s", name="psk")
            psq = psp.tile([P, T // 2], f32, tag="s", name="psq")
            for k in range(KT):
                for ps, w_sb in ((psk, wk_sb), (psq, wq_sb)):
                    for c0 in range(0, T // 2, 512):
                        nc.tensor.matmul(
                            ps[:, c0:c0 + 512],
                            lhsT=w_sb[:, k, 0:P],
                            rhs=xT_sb[:, k, c0:c0 + 512],
                            start=(k == 0), stop=(k == KT - 1),
                        )
            for c0 in range(0, T // 2, 512):
                nc.vector.tensor_copy(kT_sb[:, 0, c0:c0 + 512],
                                      psk[:, c0:c0 + 512])
                nc.vector.tensor_copy(qT_sb[:, 0, c0:c0 + 512],
                                      psq[:, c0:c0 + 512])

        def proj_full_ve(w_sb, dst, m, jh):
            """1024-wide projection as two 1-bank ve tiles (lead only)."""
            for qq in (2 * jh, 2 * jh + 1):
                ps = pse.tile([P, 512], f32, tag="ve")
                for k in range(KT):
                    nc.tensor.matmul(
                        ps,
                        lhsT=w_sb[:, k, m * P:(m + 1) * P],
                        rhs=xT_sb[:, k, qq * 512:(qq + 1) * 512],
                        start=(k == 0), stop=(k == KT - 1),
                    )
                nc.vector.tensor_copy(dst[:, m, qq * 512:(qq + 1) * 512], ps)

        def proj_units(w_sb, dst, m, qq):
            """A 512-wide projection sliced into 8 single-matmul fill units
            (~320ns each) so it slots into per-kt PE slack without ever
            stalling the exp stream."""
            state = {}

            def unit():
                k = state.get("k", 0)
                if k == 0:
                    state["ps"] = pse.tile([P, 512], f32, tag="ve",
                                           name="psu")
                nc.tensor.matmul(
                    state["ps"],
                    lhsT=w_sb[:, k, m * P:(m + 1) * P],
                    rhs=xT_sb[:, k, qq * 512:(qq + 1) * 512],
                    start=(k == 0), stop=(k == KT - 1),
                )
                if k == KT - 1:
                    nc.vector.tensor_copy(
                        dst[:, m, qq * 512:(qq + 1) * 512], state["ps"])
                state["k"] = k + 1

            return [unit] * KT

        def proj_v_tile(t):
            ps = pse.tile([P, C], f32, tag="ve")
            for k in range(KT):
                nc.tensor.matmul(
                    ps,
                    lhsT=xT_sb[:, k, t * P:(t + 1) * P],
                    rhs=wv_sb[:, k, :],
                    start=(k == 0), stop=(k == KT - 1),
                )
            nc.vector.tensor_copy(
                va_sb[:, t, :, 0:DH],
                ps.rearrange("p (h d) -> p h d", h=HC))

        def normalize_bounce(prev):
            """DMA-bounce normalize of the PREVIOUS block, emitted inside
            the next block so the DRAM round-trip latency is hidden.  The
            u-copy frees the po banks within ~1us of the block start."""
            m, jq, pos = prev
            q0 = jq * JW
            for hh in range(2):
                u = u_pool.tile([DH + 1, JW], f32, tag="u")
                nc.vector.tensor_copy(u, pos[hh])
                r_dr = nrm_d.tile([1, JW], f32, tag="rd")
                eng = nc.sync if hh == 0 else nc.gpsimd
                eng.dma_start(out=r_dr, in_=u[DH:DH + 1, :])
                lbc = nrm_b.tile([DH, JW], f32, tag="lbc")
                eng.dma_start(out=lbc, in_=r_dr.to_broadcast((DH, JW)))
                rbc = nrm_b.tile([DH, JW], f32, tag="rbc")
                nc.vector.reciprocal_approx_fast(rbc, lbc)
                if hh == 0:
                    nc.vector.tensor_mul(
                        oT_sb[0:DH, m, q0:q0 + JW], u[0:DH, :], rbc)
                else:
                    st = nrm_b.tile([DH, JW], dtx, tag="st")
                    nc.vector.tensor_mul(st, u[0:DH, :], rbc)
                    eng.dma_start(out=oT_sb[DH:P, m, q0:q0 + JW], in_=st)

        def normalize_tail(prev):
            """Fast-path normalize for the last block: reciprocal row
            broadcast via a K=1 PE matmul, scalar engine assists."""
            m, jq, pos = prev
            q0 = jq * JW
            for hh in range(2):
                uv = u_pool.tile([DH, JW], f32, tag="u2")
                ud = u_pool.tile([1, JW], f32, tag="ud")
                (nc.vector.tensor_copy if hh == 0 else nc.scalar.copy)(
                    uv, pos[hh][0:DH, :])
                (nc.scalar.copy if hh == 0 else nc.vector.tensor_copy)(
                    ud, pos[hh][DH:DH + 1, :])
                rr = nrm_b.tile([1, JW], f32, tag="rr")
                nc.vector.reciprocal_approx_fast(rr, ud)
                rb = pse.tile([DH, JW], f32, tag="ve")
                nc.tensor.matmul(rb, lhsT=ones_r, rhs=rr,
                                 start=True, stop=True)
                if hh == 0:
                    nc.vector.tensor_mul(
                        oT_sb[0:DH, m, q0:q0 + JW], uv, rb)
                else:
                    st = nrm_b.tile([DH, JW], dtx, tag="st")
                    nc.vector.tensor_mul(st, uv, rb)
                    nc.gpsimd.dma_start(
                        out=oT_sb[DH:P, m, q0:q0 + JW], in_=st)

        def attention(m, jq, fills=None, pre_kt=None, norm_prev=None):
            """Head pair m, query block jq: scores (row-tiled pair), exp
            (one call for both heads), attnV with ones-column denominator.
            fills: {kt: emitter} of background PE work slotted after the
            scores of that kt.  Returns (m, jq, po-tiles) for deferred
            normalization inside the next block."""
            q0 = jq * JW
            pos = [pso.tile([DH + 1, JW], f32, tag=f"o{hh}", name=f"po{hh}")
                   for hh in range(2)]
            for kt in range(TT):
                ps = psp.tile([P, 2 * JW], f32, tag="s")
                for hh in range(2):
                    nc.tensor.matmul(
                        ps[:, hh * JW:(hh + 1) * JW],
                        lhsT=kT_sb[hh * DH:(hh + 1) * DH, m,
                                   kt * P:(kt + 1) * P],
                        rhs=qT_sb[hh * DH:(hh + 1) * DH, m, q0:q0 + JW],
                        start=True, stop=True,
                    )
                if kt == 0 and norm_prev is not None:
                    normalize_bounce(norm_prev)
                for u in (fills.get(kt, ()) if fills else ()):
                    u()
                if pre_kt is not None:
                    pre_kt(kt)
                p_t = p_pool.tile([P, 2 * JW], dtx, tag="p")
                nc.scalar.activation(p_t, ps, Exp)
                for hh in range(2):
                    nc.tensor.matmul(
                        pos[hh],
                        lhsT=va_sb[:, kt, 2 * m + hh, 0:DH + 1],
                        rhs=p_t[:, hh * JW:(hh + 1) * JW],
                        start=(kt == 0), stop=(kt == TT - 1),
                    )
            return (m, jq, pos)

        def emerge_units(t, tail=False):
            """out[t,:] as two ~1.1us fill units (one per 512-chunk)."""
            state = {}

            def unit():
                ci = state.get("ci", 0)
                if ci == 0:
                    state["ob"] = out_pool.tile([P, D], f32, tag="ob",
                                                name="ob")
                ob = state["ob"]
                c0 = ci * 512
                pe = pse.tile([P, 512], f32, tag="ve", name="pe")
                for mm in range(MT):
                    nc.tensor.matmul(
                        pe,
                        lhsT=oT_sb[:, mm, t * P:(t + 1) * P],
                        rhs=we_sb[:, mm, c0:c0 + 512],
                        start=(mm == 0), stop=(mm == MT - 1),
                    )
                if tail and ci == 1:
                    nc.scalar.copy(ob[:, c0:c0 + 512], pe)
                else:
                    nc.vector.tensor_copy(ob[:, c0:c0 + 512], pe)
                if ci == 1:
                    eng = nc.sync if t % 2 == 0 else nc.gpsimd
                    eng.dma_start(out=out[t * P:(t + 1) * P, :], in_=ob)
                state["ci"] = ci + 1

            return [unit, unit]

        def spread(units, kts):
            """Distribute fill units round-robin over kt positions."""
            fills = {}
            n, s = len(units), len(kts)
            i = 0
            for idx, kt in enumerate(kts):
                take = (n * (idx + 1)) // s - (n * idx) // s
                if take:
                    fills[kt] = units[i:i + take]
                i += take
            return fills

        # ---- emission = per-engine queue order.  Lead: k/q(pair0,jh0)
        # under the xT DMA, then k(0,jh1) + v tiles 0-9.  All m=0 blocks
        # first; pair-1 projections are single-matmul fill units spread
        # through m=0 blocks; emerges spread through m=1 blocks; each
        # block's normalize is a kt=0 fill of the NEXT block.
        proj_kq_lead()
        proj_full_ve(wk_sb, kT_sb, 0, 1)
        for t in range(10):
            proj_v_tile(t)
        pv = attention(0, 0, fills={
            kt: [lambda t=kt: proj_v_tile(t)] for kt in range(10, TT)})
        pv = attention(0, 1, norm_prev=pv, fills=spread(
            proj_units(wq_sb, qT_sb, 0, 2) + proj_units(wq_sb, qT_sb, 0, 3),
            range(TT)))
        pv = attention(0, 2, norm_prev=pv, fills=spread(
            proj_units(wk_sb, kT_sb, 1, 0) + proj_units(wk_sb, kT_sb, 1, 1)
            + proj_units(wq_sb, qT_sb, 1, 0), range(TT)))
        pv = attention(0, 3, norm_prev=pv, fills=spread(
            proj_units(wk_sb, kT_sb, 1, 2) + proj_units(wk_sb, kT_sb, 1, 3)
            + proj_units(wq_sb, qT_sb, 1, 1), range(TT)))
        pv = attention(1, 0, norm_prev=pv, fills=spread(
            proj_units(wq_sb, qT_sb, 1, 2) + proj_units(wq_sb, qT_sb, 1, 3),
            range(8)))
        pv = attention(1, 1, norm_prev=pv, fills=spread(
            emerge_units(0) + emerge_units(1) + emerge_units(2)
            + emerge_units(3), range(7, 15)))
        pv = attention(1, 2, norm_prev=pv, fills=spread(
            emerge_units(4) + emerge_units(5) + emerge_units(6)
            + emerge_units(7), range(7, 15)))
        pv = attention(1, 3, norm_prev=pv, fills=spread(
            emerge_units(8) + emerge_units(9) + emerge_units(10)
            + emerge_units(11), range(6, 14)))
        normalize_tail(pv)
        for t in range(12, 16):
            for u in emerge_units(t, tail=True):
                u()

    nc.compile()
    return nc


def prep_inputs(x, Wq, Wk, Wv, We, beta, input_valve, output_valve, chi,
                cfg=CFG):
    """Host-side prep: fold chamber into We, fold 1/scale into Wq, shard."""
    x = np.asarray(x, np.float32)
    Wq = np.asarray(Wq, np.float32)
    Wk = np.asarray(Wk, np.float32)
    Wv = np.asarray(Wv, np.float32)
    We = np.asarray(We, np.float32)

    def sig(v):
        return 1.0 / (1.0 + np.exp(-np.asarray(v, np.float64)))

    b = sig(beta)
    iv = sig(input_valve)
    ov = sig(output_valve)
    g = np.tanh(np.asarray(chi, np.float64))
    ang = math.pi * b
    ca, sa = np.cos(ang), np.sin(ang)
    half = DH // 2

    We64 = We.astype(np.float64)
    WeP = np.empty((D, D), np.float64)
    for h in range(H):
        L = np.zeros((DH, DH))
        idx = np.arange(half)
        L[idx, idx] = ca[h]
        L[idx, half + idx] = -sa[h]
        L[half + idx, idx] = sa[h]
        L[half + idx, half + idx] = ca[h]
        L *= ov[h] * g[h] * iv[h]
        WeP[:, h * DH:(h + 1) * DH] = We64[:, h * DH:(h + 1) * DH] @ L

    dt_x = _np_dt(cfg["dt"])
    WqT = np.ascontiguousarray((Wq.astype(np.float64) / SCALE).T, dt_x)
    WkT = np.ascontiguousarray(Wk.T, dt_x)
    WvT = np.ascontiguousarray(Wv.T, dt_x)
    WeT = np.ascontiguousarray(WeP.T, dt_x)   # [c, dout]

    in_maps = []
    for core in range(NCORES):
        bidx, grp = divmod(core, H // HC)
        cols = slice(grp * C, (grp + 1) * C)
        in_maps.append({
            "xt": np.ascontiguousarray(x[bidx].T.astype(dt_x)),
            "wq": np.ascontiguousarray(WqT[:, cols]),
            "wk": np.ascontiguousarray(WkT[:, cols]),
            "wv": np.ascontiguousarray(WvT[:, cols]),
            "we": np.ascontiguousarray(WeT[cols, :]),
        })
    return in_maps


def kernel(**inputs):
    global LAST_EXEC_NS
    import os
    if "nc" not in _CACHE:
        _CACHE["nc"] = build_nc()
    nc = _CACHE["nc"]
    in_maps = prep_inputs(**inputs)

    from concourse.bass_utils import run_bass_kernel_spmd
    trace = bool(os.environ.get("CIRC_TRACE"))
    res = run_bass_kernel_spmd(nc, in_maps, list(range(NCORES)), trace=trace)
    LAST_EXEC_NS = res.exec_time_ns
    _CACHE["last_results"] = res

    B = 2
    outp = np.zeros((B, T, D), np.float32)
    per_batch = NCORES // B
    for core in range(NCORES):
        outp[core // per_batch] += res.results[core]["out"]
    return outp


# revision 20
# speedup vs baseline: 1.1238x; 1.0075x over previous
"""Trainium2 Bass kernel for nn_CircumpunctAttention.

Full inputs in, full output out. Internally: data-parallel over batch (2) x
tensor-parallel over heads (4 head-groups of 4 heads) = 8 NeuronCores.

Per core: multi-head attention on 4 heads (= 2 pairs).  Head pair m is
stacked on the partition axis (head 2m at partitions 0-63, head 2m+1 at
64-127), so the two K=64 scores matmuls of a pair run CONCURRENTLY in the
PE array as row-tiles (tile_position auto-derived from base_partition) and
one exp activation per (pair, query-block, key-tile) covers both heads at
free dim 1024.  The ScalarE exp stream (16.8M exps at 1 elem/lane/cycle)
is the throughput wall; the scalar queue carries nothing but exp, and all
other work (projections, attnV, emerge, normalize) is scheduled into the
PE/DVE slack under it.

attnV uses the ones-column trick (lhsT = [v | 1], M=65) so the softmax
denominator accumulates in PSUM partition 64 alongside the weighted sum.
The reciprocal of the denominator is broadcast across partitions with a
tiny K=1 PE matmul against a constant ones row (no DRAM round-trip).

PSUM budget (8 banks): scores 2 bufs x [128,1024]f32 (4) + two [65,512]
attnV accumulators (2) + a [128,512] pool shared by v-proj / projection
quarters / emerge / normalize-broadcast (2).  Background projections are
emitted as 1-bank "quarters" so they never stall the scores double-buffer.

The per-head "aperture chamber" (valves, rotation, tanh gate) is folded
into We host-side in float64.  Softmax max-subtraction skipped: scores are
bounded (|s| < ~7).  All matmul operands fp16.
"""

import math
from contextlib import ExitStack
import numpy as np

# ---------------------------------------------------------------- constants
P = 128          # partitions
T = 2048         # sequence length
D = 1024         # model dim
H = 16           # total heads
DH = 64          # head dim
HC = 4           # heads per core
C = HC * DH      # channels per core (256)
KT = D // P      # 8 contraction tiles over model dim
TT = T // P      # 16 tiles over sequence
MT = C // P      # 2 partition tiles (= head pairs) per core
NJ = 4           # query blocks
JW = T // NJ     # query block width (512)
NCORES = 8
SCALE = 8.0      # sqrt(dh * conv_factor), conv_factor = 1/phi^0 = 1

CFG = {
    "dt": "float16",   # matmul operand dtype (storage); accum stays f32
    "attn_ksplit": False,  # two K=64 same-bank halves per attnV MM: raises
                           # a HW error (concurrent row-tile drain to one
                           # PSUM bank) — keep False
}

LAST_EXEC_NS = None
_CACHE = {}


def _np_dt(name):
    if name == "bfloat16":
        import ml_dtypes
        return np.dtype(ml_dtypes.bfloat16)
    return np.dtype(name)


def build_nc(cfg=CFG):
    """Build + compile the single-core SPMD program."""
    import concourse.bass as bass
    import concourse.mybir as mybir
    import concourse.tile as tile
    from concourse import bacc

    dt = mybir.dt
    f32 = dt.float32
    dtx = getattr(dt, cfg["dt"])

    nc = bacc.Bacc("TRN2", target_bir_lowering=False, debug=False,
                   enable_asserts=False)

    xT = nc.dram_tensor("xt", [D, T], dtx, kind="ExternalInput").ap()
    wq = nc.dram_tensor("wq", [D, C], dtx, kind="ExternalInput").ap()
    wk = nc.dram_tensor("wk", [D, C], dtx, kind="ExternalInput").ap()
    wv = nc.dram_tensor("wv", [D, C], dtx, kind="ExternalInput").ap()
    we = nc.dram_tensor("we", [C, D], dtx, kind="ExternalInput").ap()
    out = nc.dram_tensor("out", [T, D], f32, kind="ExternalOutput").ap()

    Exp = mybir.ActivationFunctionType.Exp

    with tile.TileContext(nc) as tc, ExitStack() as ctx:
        cp = ctx.enter_context(tc.tile_pool(name="const", bufs=1))
        psp = ctx.enter_context(tc.tile_pool(name="psum", bufs=2,
                                             space="PSUM"))
        pso = ctx.enter_context(tc.tile_pool(name="psum_o", bufs=1,
                                             space="PSUM"))
        pse = ctx.enter_context(tc.tile_pool(name="psum_ve", bufs=2,
                                             space="PSUM"))
        p_pool = ctx.enter_context(tc.tile_pool(name="pp", bufs=4))
        u_pool = ctx.enter_context(tc.tile_pool(name="usb", bufs=2))
        nrm_b = ctx.enter_context(tc.tile_pool(name="nrm_b", bufs=2))
        nrm_d = ctx.enter_context(tc.tile_pool(name="nrm_d", bufs=2,
                                               space="DRAM"))
        out_pool = ctx.enter_context(tc.tile_pool(name="oute", bufs=2))

        xT_sb = cp.tile([P, KT, T], dtx)
        wq_sb = cp.tile([P, KT, C], dtx)
        wk_sb = cp.tile([P, KT, C], dtx)
        wv_sb = cp.tile([P, KT, C], dtx)
        we_sb = cp.tile([P, MT, D], dtx)
        qT_sb = cp.tile([P, MT, T], dtx)
        kT_sb = cp.tile([P, MT, T], dtx)
        # v augmented with a ones column at index DH (softmax denominator);
        # padded to 68 for 4-byte-aligned strides. cols 65-67 never read.
        va_sb = cp.tile([P, TT, HC, 68], dtx)
        oT_sb = cp.tile([P, MT, T], dtx)
        ones_r = cp.tile([1, DH], f32)       # lhsT of the bcast matmul
        dummy = cp.tile([1, 16], f32)
        nc.vector.memset(va_sb[:, :, :, DH:DH + 1], 1.0)
        nc.vector.memset(ones_r, 1.0)
        nc.vector.memset(dummy, 0.0)
        # warm the exp table while input DMAs run
        nc.scalar.activation(dummy, dummy, Exp)

        # ---- loads.  DMA engines fair-share bandwidth across in-flight
        # transfers, so the critical-path loads (wk/wq + xT chunks) are
        # issued strictly first; xT is chunked 6 ways across the three
        # DMA-capable queues (sync, gpsimd, scalar).
        nc.sync.dma_start(out=wk_sb, in_=wk.rearrange("(k p) c -> p k c", p=P))
        nc.gpsimd.dma_start(out=wq_sb,
                            in_=wq.rearrange("(k p) c -> p k c", p=P))
        nc.scalar.dma_start(out=wv_sb,
                            in_=wv.rearrange("(k p) c -> p k c", p=P))
        dq = (nc.sync, nc.gpsimd, nc.scalar)
        xbounds = [0, 1, 2, 3, 4, 6, 8]
        for ch in range(6):
            a, b = xbounds[ch], xbounds[ch + 1]
            dq[ch % 3].dma_start(
                out=xT_sb[:, a:b, :],
                in_=xT[a * P:b * P, :].rearrange("(k p) t -> p k t", p=P))
        nc.sync.dma_start(out=we_sb,
                            in_=we.rearrange("(m p) d -> p m d", p=P))

        def proj_kq_lead():
            """k and q projections for pair 0's first query/key half,
            interleaved per k-tile so both finish right after the xT DMA;
            casts split in halves to release the first scores earlier."""
            psk = psp.tile([P, T // 2], f32, tag="s", name="psk")
            psq = psp.tile([P, T // 2], f32, tag="s", name="psq")
            for k in range(KT):
                for ps, w_sb in ((psk, wk_sb), (psq, wq_sb)):
                    for c0 in range(0, T // 2, 512):
                        nc.tensor.matmul(
                            ps[:, c0:c0 + 512],
                            lhsT=w_sb[:, k, 0:P],
                            rhs=xT_sb[:, k, c0:c0 + 512],
                            start=(k == 0), stop=(k == KT - 1),
                        )
            for c0 in range(0, T // 2, 512):
                nc.vector.tensor_copy(kT_sb[:, 0, c0:c0 + 512],
                                      psk[:, c0:c0 + 512])
                nc.vector.tensor_copy(qT_sb[:, 0, c0:c0 + 512],
                                      psq[:, c0:c0 + 512])

        def proj_full_ve(w_sb, dst, m, jh):
            """1024-wide projection as two 1-bank ve tiles (lead only)."""
            for qq in (2 * jh, 2 * jh + 1):
                ps = pse.tile([P, 512], f32, tag="ve")
                for k in range(KT):
                    nc.tensor.matmul(
                        ps,
                        lhsT=w_sb[:, k, m * P:(m + 1) * P],
                        rhs=xT_sb[:, k, qq * 512:(qq + 1) * 512],
                        start=(k == 0), stop=(k == KT - 1),
                    )
                nc.vector.tensor_copy(dst[:, m, qq * 512:(qq + 1) * 512], ps)

        def proj_units(w_sb, dst, m, qq):
            """A 512-wide projection sliced into 8 single-matmul fill units
            (~320ns each) so it slots into per-kt PE slack without ever
            stalling the exp stream."""
            state = {}

            def unit():
                k = state.get("k", 0)
                if k == 0:
                    state["ps"] = pse.tile([P, 512], f32, tag="ve",
                                           name="psu")
                nc.tensor.matmul(
                    state["ps"],
                    lhsT=w_sb[:, k, m * P:(m + 1) * P],
                    rhs=xT_sb[:, k, qq * 512:(qq + 1) * 512],
                    start=(k == 0), stop=(k == KT - 1),
                )
                if k == KT - 1:
                    nc.vector.tensor_copy(
                        dst[:, m, qq * 512:(qq + 1) * 512], state["ps"])
                state["k"] = k + 1

            return [unit] * KT

        def proj_v_tile(t):
            ps = pse.tile([P, C], f32, tag="ve")
            for k in range(KT):
                nc.tensor.matmul(
                    ps,
                    lhsT=xT_sb[:, k, t * P:(t + 1) * P],
                    rhs=wv_sb[:, k, :],
                    start=(k == 0), stop=(k == KT - 1),
                )
            nc.vector.tensor_copy(
                va_sb[:, t, :, 0:DH],
                ps.rearrange("p (h d) -> p h d", h=HC))

        def normalize_bounce(prev):
            """DMA-bounce normalize of the PREVIOUS block, emitted inside
            the next block so the DRAM round-trip latency is hidden.  The
            u-copy frees the po banks within ~1us of the block start."""
            m, jq, pos = prev
            q0 = jq * JW
            for hh in range(2):
                u = u_pool.tile([DH + 1, JW], f32, tag="u")
                nc.vector.tensor_copy(u, pos[hh])
                r_dr = nrm_d.tile([1, JW], f32, tag="rd")
                eng = nc.sync if hh == 0 else nc.gpsimd
                eng.dma_start(out=r_dr, in_=u[DH:DH + 1, :])
                lbc = nrm_b.tile([DH, JW], f32, tag="lbc")
                eng.dma_start(out=lbc, in_=r_dr.to_broadcast((DH, JW)))
                rbc = nrm_b.tile([DH, JW], f32, tag="rbc")
                nc.vector.reciprocal_approx_fast(rbc, lbc)
                if hh == 0:
                    nc.vector.tensor_mul(
                        oT_sb[0:DH, m, q0:q0 + JW], u[0:DH, :], rbc)
                else:
                    st = nrm_b.tile([DH, JW], dtx, tag="st")
                    nc.vector.tensor_mul(st, u[0:DH, :], rbc)
                    eng.dma_start(out=oT_sb[DH:P, m, q0:q0 + JW], in_=st)

        def normalize_tail(prev):
            """Fast-path normalize for the last block: reciprocal row
            broadcast via a K=1 PE matmul, scalar engine assists."""
            m, jq, pos = prev
            q0 = jq * JW
            for hh in range(2):
                uv = u_pool.tile([DH, JW], f32, tag="u2")
                ud = u_pool.tile([1, JW], f32, tag="ud")
                (nc.vector.tensor_copy if hh == 0 else nc.scalar.copy)(
                    uv, pos[hh][0:DH, :])
                (nc.scalar.copy if hh == 0 else nc.vector.tensor_copy)(
                    ud, pos[hh][DH:DH + 1, :])
                rr = nrm_b.tile([1, JW], f32, tag="rr")
                nc.vector.reciprocal_approx_fast(rr, ud)
                rb = pse.tile([DH, JW], f32, tag="ve")
                nc.tensor.matmul(rb, lhsT=ones_r, rhs=rr,
                                 start=True, stop=True)
                if hh == 0:
                    nc.vector.tensor_mul(
                        oT_sb[0:DH, m, q0:q0 + JW], uv, rb)
                else:
                    st = nrm_b.tile([DH, JW], dtx, tag="st")
                    nc.vector.tensor_mul(st, uv, rb)
                    nc.gpsimd.dma_start(
                        out=oT_sb[DH:P, m, q0:q0 + JW], in_=st)

        def attention(m, jq, fills=None, pre_kt=None, norm_prev=None):
            """Head pair m, query block jq: scores (row-tiled pair), exp
            (one call for both heads), attnV with ones-column denominator.
            fills: {kt: emitter} of background PE work slotted after the
            scores of that kt.  Returns (m, jq, po-tiles) for deferred
            normalization inside the next block."""
            q0 = jq * JW
            pos = [pso.tile([DH + 1, JW], f32, tag=f"o{hh}", name=f"po{hh}")
                   for hh in range(2)]
            for kt in range(TT):
                ps = psp.tile([P, 2 * JW], f32, tag="s")
                for hh in range(2):
                    nc.tensor.matmul(
                        ps[:, hh * JW:(hh + 1) * JW],
                        lhsT=kT_sb[hh * DH:(hh + 1) * DH, m,
                                   kt * P:(kt + 1) * P],
                        rhs=qT_sb[hh * DH:(hh + 1) * DH, m, q0:q0 + JW],
                        start=True, stop=True,
                    )
                if kt == 0 and norm_prev is not None:
                    normalize_bounce(norm_prev)
                for u in (fills.get(kt, ()) if fills else ()):
                    u()
                if pre_kt is not None:
                    pre_kt(kt)
                p_t = p_pool.tile([P, 2 * JW], dtx, tag="p")
                nc.scalar.activation(p_t, ps, Exp)
                for hh in range(2):
                    if cfg["attn_ksplit"]:
                        for rg in range(2):
                            nc.tensor.matmul(
                                pos[hh],
                                lhsT=va_sb[rg * DH:(rg + 1) * DH, kt,
                                           2 * m + hh, 0:DH + 1],
                                rhs=p_t[rg * DH:(rg + 1) * DH,
                                        hh * JW:(hh + 1) * JW],
                                start=(kt == 0 and rg == 0),
                                stop=(kt == TT - 1 and rg == 1),
                            )
                    else:
                        nc.tensor.matmul(
                            pos[hh],
                            lhsT=va_sb[:, kt, 2 * m + hh, 0:DH + 1],
                            rhs=p_t[:, hh * JW:(hh + 1) * JW],
                            start=(kt == 0), stop=(kt == TT - 1),
                        )
            return (m, jq, pos)

        def emerge_units(t, tail=False):
            """out[t,:] as two ~1.1us fill units (one per 512-chunk)."""
            state = {}

            def unit():
                ci = state.get("ci", 0)
                if ci == 0:
                    state["ob"] = out_pool.tile([P, D], f32, tag="ob",
                                                name="ob")
                ob = state["ob"]
                c0 = ci * 512
                pe = pse.tile([P, 512], f32, tag="ve", name="pe")
                for mm in range(MT):
                    nc.tensor.matmul(
                        pe,
                        lhsT=oT_sb[:, mm, t * P:(t + 1) * P],
                        rhs=we_sb[:, mm, c0:c0 + 512],
                        start=(mm == 0), stop=(mm == MT - 1),
                    )
                if tail and ci == 1:
                    nc.scalar.copy(ob[:, c0:c0 + 512], pe)
                else:
                    nc.vector.tensor_copy(ob[:, c0:c0 + 512], pe)
                if ci == 1:
                    eng = nc.sync if t % 2 == 0 else nc.gpsimd
                    eng.dma_start(out=out[t * P:(t + 1) * P, :], in_=ob)
                state["ci"] = ci + 1

            return [unit, unit]

        def spread(units, kts):
            """Distribute fill units round-robin over kt positions."""
            fills = {}
            n, s = len(units), len(kts)
            i = 0
            for idx, kt in enumerate(kts):
                take = (n * (idx + 1)) // s - (n * idx) // s
                if take:
                    fills[kt] = units[i:i + take]
                i += take
            return fills

        # ---- emission = per-engine queue order.  Lead: k/q(pair0,jh0)
        # under the xT DMA, then k(0,jh1) + v tiles 0-9.  All m=0 blocks
        # first; pair-1 projections are single-matmul fill units spread
        # through m=0 blocks; emerges spread through m=1 blocks; each
        # block's normalize is a kt=0 fill of the NEXT block.
        proj_kq_lead()
        proj_full_ve(wk_sb, kT_sb, 0, 1)
        for t in range(10):
            proj_v_tile(t)
        pv = attention(0, 0, fills={
            kt: [lambda t=kt: proj_v_tile(t)] for kt in range(10, TT)})
        pv = attention(0, 1, norm_prev=pv, fills=spread(
            proj_units(wq_sb, qT_sb, 0, 2) + proj_units(wq_sb, qT_sb, 0, 3),
            range(TT)))
        pv = attention(0, 2, norm_prev=pv, fills=spread(
            proj_units(wk_sb, kT_sb, 1, 0) + proj_units(wk_sb, kT_sb, 1, 1)
            + proj_units(wq_sb, qT_sb, 1, 0), range(TT)))
        pv = attention(0, 3, norm_prev=pv, fills=spread(
            proj_units(wk_sb, kT_sb, 1, 2) + proj_units(wk_sb, kT_sb, 1, 3)
            + proj_units(wq_sb, qT_sb, 1, 1), range(TT)))
        pv = attention(1, 0, norm_prev=pv, fills=spread(
            proj_units(wq_sb, qT_sb, 1, 2) + proj_units(wq_sb, qT_sb, 1, 3),
            range(8)))
        pv = attention(1, 1, norm_prev=pv, fills=spread(
            emerge_units(0) + emerge_units(1) + emerge_units(2)
            + emerge_units(3), range(7, 15)))
        pv = attention(1, 2, norm_prev=pv, fills=spread(
            emerge_units(4) + emerge_units(5) + emerge_units(6)
            + emerge_units(7), range(7, 15)))
        pv = attention(1, 3, norm_prev=pv, fills=spread(
            emerge_units(8) + emerge_units(9) + emerge_units(10)
            + emerge_units(11), range(6, 14)))
        normalize_tail(pv)
        for t in range(12, 16):
            for u in emerge_units(t, tail=True):
                u()

    nc.compile()
    return nc


def prep_inputs(x, Wq, Wk, Wv, We, beta, input_valve, output_valve, chi,
                cfg=CFG):
    """Host-side prep: fold chamber into We, fold 1/scale into Wq, shard."""
    x = np.asarray(x, np.float32)
    Wq = np.asarray(Wq, np.float32)
    Wk = np.asarray(Wk, np.float32)
    Wv = np.asarray(Wv, np.float32)
    We = np.asarray(We, np.float32)

    def sig(v):
        return 1.0 / (1.0 + np.exp(-np.asarray(v, np.float64)))

    b = sig(beta)
    iv = sig(input_valve)
    ov = sig(output_valve)
    g = np.tanh(np.asarray(chi, np.float64))
    ang = math.pi * b
    ca, sa = np.cos(ang), np.sin(ang)
    half = DH // 2

    We64 = We.astype(np.float64)
    WeP = np.empty((D, D), np.float64)
    for h in range(H):
        L = np.zeros((DH, DH))
        idx = np.arange(half)
        L[idx, idx] = ca[h]
        L[idx, half + idx] = -sa[h]
        L[half + idx, idx] = sa[h]
        L[half + idx, half + idx] = ca[h]
        L *= ov[h] * g[h] * iv[h]
        WeP[:, h * DH:(h + 1) * DH] = We64[:, h * DH:(h + 1) * DH] @ L

    dt_x = _np_dt(cfg["dt"])
    WqT = np.ascontiguousarray((Wq.astype(np.float64) / SCALE).T, dt_x)
    WkT = np.ascontiguousarray(Wk.T, dt_x)
    WvT = np.ascontiguousarray(Wv.T, dt_x)
    WeT = np.ascontiguousarray(WeP.T, dt_x)   # [c, dout]

    in_maps = []
    for core in range(NCORES):
        bidx, grp = divmod(core, H // HC)
        cols = slice(grp * C, (grp + 1) * C)
        in_maps.append({
            "xt": np.ascontiguousarray(x[bidx].T.astype(dt_x)),
            "wq": np.ascontiguousarray(WqT[:, cols]),
            "wk": np.ascontiguousarray(WkT[:, cols]),
            "wv": np.ascontiguousarray(WvT[:, cols]),
            "we": np.ascontiguousarray(WeT[cols, :]),
        })
    return in_maps


def kernel(**inputs):
    global LAST_EXEC_NS
    import os
    if "nc" not in _CACHE:
        _CACHE["nc"] = build_nc()
    nc = _CACHE["nc"]
    in_maps = prep_inputs(**inputs)

    from concourse.bass_utils import run_bass_kernel_spmd
    trace = bool(os.environ.get("CIRC_TRACE"))
    res = run_bass_kernel_spmd(nc, in_maps, list(range(NCORES)), trace=trace)
    LAST_EXEC_NS = res.exec_time_ns
    _CACHE["last_results"] = res

    B = 2
    outp = np.zeros((B, T, D), np.float32)
    per_batch = NCORES // B
    for core in range(NCORES):
        outp[core // per_batch] += res.results[core]["out"]
    return outp


# revision 32
# speedup vs baseline: 1.1310x; 1.0063x over previous
"""Trainium2 Bass kernel for nn_CircumpunctAttention.

Full inputs in, full output out. Internally: data-parallel over batch (2) x
tensor-parallel over heads (4 head-groups of 4 heads) = 8 NeuronCores.

Per core: multi-head attention on 4 heads (= 2 pairs).  Head pair m is
stacked on the partition axis (head 2m at partitions 0-63, head 2m+1 at
64-127), so the two K=64 scores matmuls of a pair run CONCURRENTLY in the
PE array as row-tiles (tile_position auto-derived from base_partition) and
one exp activation per (pair, query-block, key-tile) covers both heads at
free dim 1024.  The ScalarE exp stream (16.8M exps at 1 elem/lane/cycle)
is the throughput wall; the scalar queue carries nothing but exp, and all
other work (projections, attnV, emerge, normalize) is scheduled into the
PE/DVE slack under it.

attnV uses the ones-column trick (lhsT = [v | 1], M=65) so the softmax
denominator accumulates in PSUM partition 64 alongside the weighted sum.
The reciprocal of the denominator is broadcast across partitions with a
tiny K=1 PE matmul against a constant ones row (no DRAM round-trip).

PSUM budget (8 banks): scores 2 bufs x [128,1024]f32 (4) + two [65,512]
attnV accumulators (2) + a [128,512] pool shared by v-proj / projection
quarters / emerge / normalize-broadcast (2).  Background projections are
emitted as 1-bank "quarters" so they never stall the scores double-buffer.

The per-head "aperture chamber" (valves, rotation, tanh gate) is folded
into We host-side in float64.  Softmax max-subtraction skipped: scores are
bounded (|s| < ~7).  All matmul operands fp16.
"""

import math
from contextlib import ExitStack
import numpy as np

# ---------------------------------------------------------------- constants
P = 128          # partitions
T = 2048         # sequence length
D = 1024         # model dim
H = 16           # total heads
DH = 64          # head dim
HC = 4           # heads per core
C = HC * DH      # channels per core (256)
KT = D // P      # 8 contraction tiles over model dim
TT = T // P      # 16 tiles over sequence
MT = C // P      # 2 partition tiles (= head pairs) per core
NJ = 4           # query blocks
JW = T // NJ     # query block width (512)
NCORES = 8
SCALE = 8.0      # sqrt(dh * conv_factor), conv_factor = 1/phi^0 = 1

CFG = {
    "dt": "float16",   # matmul operand dtype (storage); accum stays f32
    "attn_ksplit": False,  # two K=64 same-bank halves per attnV MM: raises
                           # a HW error (concurrent row-tile drain to one
                           # PSUM bank) — keep False
}

LAST_EXEC_NS = None
_CACHE = {}


def _np_dt(name):
    if name == "bfloat16":
        import ml_dtypes
        return np.dtype(ml_dtypes.bfloat16)
    return np.dtype(name)


def build_nc(cfg=CFG):
    """Build + compile the single-core SPMD program."""
    import concourse.bass as bass
    import concourse.mybir as mybir
    import concourse.tile as tile
    from concourse import bacc

    dt = mybir.dt
    f32 = dt.float32
    dtx = getattr(dt, cfg["dt"])

    nc = bacc.Bacc("TRN2", target_bir_lowering=False, debug=False,
                   enable_asserts=False)

    xT = nc.dram_tensor("xt", [D, T], dtx, kind="ExternalInput").ap()
    wq = nc.dram_tensor("wq", [D, C], dtx, kind="ExternalInput").ap()
    wk = nc.dram_tensor("wk", [D, C], dtx, kind="ExternalInput").ap()
    wv = nc.dram_tensor("wv", [D, C], dtx, kind="ExternalInput").ap()
    we = nc.dram_tensor("we", [C, D], dtx, kind="ExternalInput").ap()
    out = nc.dram_tensor("out", [T, D], f32, kind="ExternalOutput").ap()

    Exp = mybir.ActivationFunctionType.Exp

    with tile.TileContext(nc) as tc, ExitStack() as ctx:
        cp = ctx.enter_context(tc.tile_pool(name="const", bufs=1))
        psp = ctx.enter_context(tc.tile_pool(name="psum", bufs=2,
                                             space="PSUM"))
        pso = ctx.enter_context(tc.tile_pool(name="psum_o", bufs=1,
                                             space="PSUM"))
        pse = ctx.enter_context(tc.tile_pool(name="psum_ve", bufs=2,
                                             space="PSUM"))
        p_pool = ctx.enter_context(tc.tile_pool(name="pp", bufs=6))
        u_pool = ctx.enter_context(tc.tile_pool(name="usb", bufs=2))
        nrm_b = ctx.enter_context(tc.tile_pool(name="nrm_b", bufs=2))
        nrm_d = ctx.enter_context(tc.tile_pool(name="nrm_d", bufs=2,
                                               space="DRAM"))
        out_pool = ctx.enter_context(tc.tile_pool(name="oute", bufs=2))

        xT_sb = cp.tile([P, KT, T], dtx)
        wq_sb = cp.tile([P, KT, C], dtx)
        wk_sb = cp.tile([P, KT, C], dtx)
        wv_sb = cp.tile([P, KT, C], dtx)
        we_sb = cp.tile([P, MT, D], dtx)
        qT_sb = cp.tile([P, MT, T], dtx)
        kT_sb = cp.tile([P, MT, T], dtx)
        # v augmented with a ones column at index DH (softmax denominator);
        # padded to 68 for 4-byte-aligned strides. cols 65-67 never read.
        va_sb = cp.tile([P, TT, HC, 68], dtx)
        vT_sb = cp.tile([P, MT, T], dtx)
        oT_sb = cp.tile([P, MT, T], dtx)
        ones_r = cp.tile([1, DH], f32)       # lhsT of the bcast matmul
        dummy = cp.tile([1, 16], f32)
        nc.vector.memset(va_sb[:, :, :, DH:DH + 1], 1.0)
        nc.vector.memset(ones_r, 1.0)
        nc.vector.memset(dummy, 0.0)
        # warm the exp table while input DMAs run
        nc.scalar.activation(dummy, dummy, Exp)

        # ---- loads.  DMA engines fair-share bandwidth across in-flight
        # transfers, so the critical-path loads (wk/wq + xT chunks) are
        # issued strictly first; xT is chunked 6 ways across the three
        # DMA-capable queues (sync, gpsimd, scalar).
        nc.sync.dma_start(out=wk_sb, in_=wk.rearrange("(k p) c -> p k c", p=P))
        nc.gpsimd.dma_start(out=wq_sb,
                            in_=wq.rearrange("(k p) c -> p k c", p=P))
        dq = (nc.sync, nc.gpsimd, nc.scalar)
        xbounds = [0, 1, 2, 3, 4, 6, 8]
        for ch in range(6):
            a, b = xbounds[ch], xbounds[ch + 1]
            dq[ch % 3].dma_start(
                out=xT_sb[:, a:b, :],
                in_=xT[a * P:b * P, :].rearrange("(k p) t -> p k t", p=P))
        nc.scalar.dma_start(out=wv_sb,
                            in_=wv.rearrange("(k p) c -> p k c", p=P))
        nc.sync.dma_start(out=we_sb,
                            in_=we.rearrange("(m p) d -> p m d", p=P))

        def proj_kq_lead():
            """k and q projections for pair 0's first query/key half,
            interleaved per k-tile so both finish right after the xT DMA;
            casts split in halves to release the first scores earlier."""
            psk = psp.tile([P, T // 2], f32, tag="s", name="psk")
            psq = psp.tile([P, T // 2], f32, tag="s", name="psq")
            for k in range(KT):
                for ps, w_sb in ((psk, wk_sb), (psq, wq_sb)):
                    for c0 in range(0, T // 2, 512):
                        nc.tensor.matmul(
                            ps[:, c0:c0 + 512],
                            lhsT=w_sb[:, k, 0:P],
                            rhs=xT_sb[:, k, c0:c0 + 512],
                            start=(k == 0), stop=(k == KT - 1),
                        )
            for c0 in range(0, T // 2, 512):
                nc.vector.tensor_copy(kT_sb[:, 0, c0:c0 + 512],
                                      psk[:, c0:c0 + 512])
                nc.vector.tensor_copy(qT_sb[:, 0, c0:c0 + 512],
                                      psq[:, c0:c0 + 512])

        def proj_full_ve(w_sb, dst, m, jh):
            """1024-wide projection as two 1-bank ve tiles (lead only)."""
            for qq in (2 * jh, 2 * jh + 1):
                ps = pse.tile([P, 512], f32, tag="ve")
                for k in range(KT):
                    nc.tensor.matmul(
                        ps,
                        lhsT=w_sb[:, k, m * P:(m + 1) * P],
                        rhs=xT_sb[:, k, qq * 512:(qq + 1) * 512],
                        start=(k == 0), stop=(k == KT - 1),
                    )
                nc.vector.tensor_copy(dst[:, m, qq * 512:(qq + 1) * 512], ps)

        def proj_units(w_sb, dst, m, qq):
            """A 512-wide projection sliced into 8 single-matmul fill units
            (~320ns each) so it slots into per-kt PE slack without ever
            stalling the exp stream."""
            state = {}

            def unit():
                k = state.get("k", 0)
                if k == 0:
                    state["ps"] = pse.tile([P, 512], f32, tag="ve",
                                           name="psu")
                nc.tensor.matmul(
                    state["ps"],
                    lhsT=w_sb[:, k, m * P:(m + 1) * P],
                    rhs=xT_sb[:, k, qq * 512:(qq + 1) * 512],
                    start=(k == 0), stop=(k == KT - 1),
                )
                if k == KT - 1:
                    nc.vector.tensor_copy(
                        dst[:, m, qq * 512:(qq + 1) * 512], state["ps"])
                state["k"] = k + 1

            return [unit] * KT

        def proj_v_tile(t):
            ps = pse.tile([P, C], f32, tag="ve")
            for k in range(KT):
                nc.tensor.matmul(
                    ps,
                    lhsT=xT_sb[:, k, t * P:(t + 1) * P],
                    rhs=wv_sb[:, k, :],
                    start=(k == 0), stop=(k == KT - 1),
                )
            nc.vector.tensor_copy(
                va_sb[:, t, :, 0:DH],
                ps.rearrange("p (h d) -> p h d", h=HC))

        def normalize_bounce(prev):
            """DMA-bounce normalize of the PREVIOUS block, emitted inside
            the next block so the DRAM round-trip latency is hidden.  The
            u-copy frees the po banks within ~1us of the block start."""
            m, jq, pos = prev
            q0 = jq * JW
            for hh in range(2):
                u = u_pool.tile([DH + 1, JW], f32, tag="u")
                nc.vector.tensor_copy(u, pos[hh])
                r_dr = nrm_d.tile([1, JW], f32, tag="rd")
                eng = nc.sync if hh == 0 else nc.gpsimd
                eng.dma_start(out=r_dr, in_=u[DH:DH + 1, :])
                lbc = nrm_b.tile([DH, JW], f32, tag="lbc")
                eng.dma_start(out=lbc, in_=r_dr.to_broadcast((DH, JW)))
                rbc = nrm_b.tile([DH, JW], f32, tag="rbc")
                nc.vector.reciprocal_approx_fast(rbc, lbc)
                if hh == 0:
                    nc.vector.tensor_mul(
                        oT_sb[0:DH, m, q0:q0 + JW], u[0:DH, :], rbc)
                else:
                    st = nrm_b.tile([DH, JW], dtx, tag="st")
                    nc.vector.tensor_mul(st, u[0:DH, :], rbc)
                    eng.dma_start(out=oT_sb[DH:P, m, q0:q0 + JW], in_=st)

        def normalize_tail(prev):
            """Fast-path normalize for the last block: reciprocal row
            broadcast via a K=1 PE matmul, scalar engine assists."""
            m, jq, pos = prev
            q0 = jq * JW
            for hh in range(2):
                uv = u_pool.tile([DH, JW], f32, tag="u2")
                ud = u_pool.tile([1, JW], f32, tag="ud")
                (nc.vector.tensor_copy if hh == 0 else nc.scalar.copy)(
                    uv, pos[hh][0:DH, :])
                (nc.scalar.copy if hh == 0 else nc.vector.tensor_copy)(
                    ud, pos[hh][DH:DH + 1, :])
                rr = nrm_b.tile([1, JW], f32, tag="rr")
                nc.vector.reciprocal_approx_fast(rr, ud)
                rb = pse.tile([DH, JW], f32, tag="ve")
                nc.tensor.matmul(rb, lhsT=ones_r, rhs=rr,
                                 start=True, stop=True)
                if hh == 0:
                    nc.vector.tensor_mul(
                        oT_sb[0:DH, m, q0:q0 + JW], uv, rb)
                else:
                    st = nrm_b.tile([DH, JW], dtx, tag="st")
                    nc.vector.tensor_mul(st, uv, rb)
                    nc.gpsimd.dma_start(
                        out=oT_sb[DH:P, m, q0:q0 + JW], in_=st)

        def attention(m, jq, fills=None, pre_kt=None, norm_prev=None):
            """Head pair m, query block jq: scores (row-tiled pair), exp
            (one call for both heads), attnV with ones-column denominator.
            fills: {kt: emitter} of background PE work slotted after the
            scores of that kt.  Returns (m, jq, po-tiles) for deferred
            normalization inside the next block."""
            q0 = jq * JW
            pos = [pso.tile([DH + 1, JW], f32, tag=f"o{hh}", name=f"po{hh}")
                   for hh in range(2)]
            for kt in range(TT):
                ps = psp.tile([P, 2 * JW], f32, tag="s")
                for hh in range(2):
                    nc.tensor.matmul(
                        ps[:, hh * JW:(hh + 1) * JW],
                        lhsT=kT_sb[hh * DH:(hh + 1) * DH, m,
                                   kt * P:(kt + 1) * P],
                        rhs=qT_sb[hh * DH:(hh + 1) * DH, m, q0:q0 + JW],
                        start=True, stop=True,
                    )
                if kt == 0 and norm_prev is not None:
                    normalize_bounce(norm_prev)
                for u in (fills.get(kt, ()) if fills else ()):
                    u()
                if pre_kt is not None:
                    pre_kt(kt)
                p_t = p_pool.tile([P, 2 * JW], dtx, tag="p")
                nc.scalar.activation(p_t, ps, Exp)
                for hh in range(2):
                    if cfg["attn_ksplit"]:
                        for rg in range(2):
                            nc.tensor.matmul(
                                pos[hh],
                                lhsT=va_sb[rg * DH:(rg + 1) * DH, kt,
                                           2 * m + hh, 0:DH + 1],
                                rhs=p_t[rg * DH:(rg + 1) * DH,
                                        hh * JW:(hh + 1) * JW],
                                start=(kt == 0 and rg == 0),
                                stop=(kt == TT - 1 and rg == 1),
                            )
                    else:
                        nc.tensor.matmul(
                            pos[hh],
                            lhsT=va_sb[:, kt, 2 * m + hh, 0:DH + 1],
                            rhs=p_t[:, hh * JW:(hh + 1) * JW],
                            start=(kt == 0), stop=(kt == TT - 1),
                        )
            return (m, jq, pos)

        def emerge_units(t, tail=False):
            """out[t,:] as two ~1.1us fill units (one per 512-chunk)."""
            state = {}

            def unit():
                ci = state.get("ci", 0)
                if ci == 0:
                    state["ob"] = out_pool.tile([P, D], f32, tag="ob",
                                                name="ob")
                ob = state["ob"]
                c0 = ci * 512
                pe = pse.tile([P, 512], f32, tag="ve", name="pe")
                for mm in range(MT):
                    nc.tensor.matmul(
                        pe,
                        lhsT=oT_sb[:, mm, t * P:(t + 1) * P],
                        rhs=we_sb[:, mm, c0:c0 + 512],
                        start=(mm == 0), stop=(mm == MT - 1),
                    )
                if tail and ci == 1:
                    nc.scalar.copy(ob[:, c0:c0 + 512], pe)
                else:
                    nc.vector.tensor_copy(ob[:, c0:c0 + 512], pe)
                if ci == 1:
                    eng = nc.sync if t % 2 == 0 else nc.gpsimd
                    eng.dma_start(out=out[t * P:(t + 1) * P, :], in_=ob)
                state["ci"] = ci + 1

            return [unit, unit]

        def spread(units, kts):
            """Distribute fill units round-robin over kt positions."""
            fills = {}
            n, s = len(units), len(kts)
            i = 0
            for idx, kt in enumerate(kts):
                take = (n * (idx + 1)) // s - (n * idx) // s
                if take:
                    fills[kt] = units[i:i + take]
                i += take
            return fills

        # ---- emission = per-engine queue order.  Lead: k/q(pair0,jh0)
        # under the xT DMA, then k(0,jh1) + v tiles 0-9.  All m=0 blocks
        # first; pair-1 projections are single-matmul fill units spread
        # through m=0 blocks; emerges spread through m=1 blocks; each
        # block's normalize is a kt=0 fill of the NEXT block.
        proj_kq_lead()
        proj_full_ve(wk_sb, kT_sb, 0, 1)
        for t in range(10):
            proj_v_tile(t)
        pv = attention(0, 0, fills={
            kt: [lambda t=kt: proj_v_tile(t)] for kt in range(10, TT)})
        pv = attention(0, 1, norm_prev=pv, fills=spread(
            proj_units(wq_sb, qT_sb, 0, 2) + proj_units(wq_sb, qT_sb, 0, 3),
            range(TT)))
        pv = attention(0, 2, norm_prev=pv, fills=spread(
            proj_units(wk_sb, kT_sb, 1, 0) + proj_units(wk_sb, kT_sb, 1, 1)
            + proj_units(wq_sb, qT_sb, 1, 0), range(TT)))
        pv = attention(0, 3, norm_prev=pv, fills=spread(
            proj_units(wk_sb, kT_sb, 1, 2) + proj_units(wk_sb, kT_sb, 1, 3)
            + proj_units(wq_sb, qT_sb, 1, 1), range(TT)))
        pv = attention(1, 0, norm_prev=pv, fills=spread(
            proj_units(wq_sb, qT_sb, 1, 2) + proj_units(wq_sb, qT_sb, 1, 3),
            range(8)))
        pv = attention(1, 1, norm_prev=pv, fills=spread(
            emerge_units(0) + emerge_units(1) + emerge_units(2)
            + emerge_units(3), range(7, 15)))
        pv = attention(1, 2, norm_prev=pv, fills=spread(
            emerge_units(4) + emerge_units(5) + emerge_units(6)
            + emerge_units(7), range(7, 15)))
        pv = attention(1, 3, norm_prev=pv, fills=spread(
            emerge_units(8) + emerge_units(9) + emerge_units(10)
            + emerge_units(11), range(4, 12)))
        normalize_tail(pv)
        for t in range(12, 16):
            for u in emerge_units(t, tail=True):
                u()

    nc.compile()
    return nc


def prep_inputs(x, Wq, Wk, Wv, We, beta, input_valve, output_valve, chi,
                cfg=CFG):
    """Host-side prep: fold chamber into We, fold 1/scale into Wq, shard."""
    x = np.asarray(x, np.float32)
    Wq = np.asarray(Wq, np.float32)
    Wk = np.asarray(Wk, np.float32)
    Wv = np.asarray(Wv, np.float32)
    We = np.asarray(We, np.float32)

    def sig(v):
        return 1.0 / (1.0 + np.exp(-np.asarray(v, np.float64)))

    b = sig(beta)
    iv = sig(input_valve)
    ov = sig(output_valve)
    g = np.tanh(np.asarray(chi, np.float64))
    ang = math.pi * b
    ca, sa = np.cos(ang), np.sin(ang)
    half = DH // 2

    We64 = We.astype(np.float64)
    WeP = np.empty((D, D), np.float64)
    for h in range(H):
        L = np.zeros((DH, DH))
        idx = np.arange(half)
        L[idx, idx] = ca[h]
        L[idx, half + idx] = -sa[h]
        L[half + idx, idx] = sa[h]
        L[half + idx, half + idx] = ca[h]
        L *= ov[h] * g[h] * iv[h]
        WeP[:, h * DH:(h + 1) * DH] = We64[:, h * DH:(h + 1) * DH] @ L

    dt_x = _np_dt(cfg["dt"])
    WqT = np.ascontiguousarray((Wq.astype(np.float64) / SCALE).T, dt_x)
    WkT = np.ascontiguousarray(Wk.T, dt_x)
    WvT = np.ascontiguousarray(Wv.T, dt_x)
    WeT = np.ascontiguousarray(WeP.T, dt_x)   # [c, dout]

    in_maps = []
    for core in range(NCORES):
        bidx, grp = divmod(core, H // HC)
        cols = slice(grp * C, (grp + 1) * C)
        in_maps.append({
            "xt": np.ascontiguousarray(x[bidx].T.astype(dt_x)),
            "wq": np.ascontiguousarray(WqT[:, cols]),
            "wk": np.ascontiguousarray(WkT[:, cols]),
            "wv": np.ascontiguousarray(WvT[:, cols]),
            "we": np.ascontiguousarray(WeT[cols, :]),
        })
    return in_maps


def kernel(**inputs):
    global LAST_EXEC_NS
    import os
    if "nc" not in _CACHE:
        _CACHE["nc"] = build_nc()
    nc = _CACHE["nc"]
    in_maps = prep_inputs(**inputs)

    from concourse.bass_utils import run_bass_kernel_spmd
    trace = bool(os.environ.get("CIRC_TRACE"))
    res = run_bass_kernel_spmd(nc, in_maps, list(range(NCORES)), trace=trace)
    LAST_EXEC_NS = res.exec_time_ns
    _CACHE["last_results"] = res

    B = 2
    outp = np.zeros((B, T, D), np.float32)
    per_batch = NCORES // B
    for core in range(NCORES):
        outp[core // per_batch] += res.results[core]["out"]
    return outp


# revision 36
# speedup vs baseline: 1.1445x; 1.0119x over previous
"""Trainium2 Bass kernel for nn_CircumpunctAttention.

Full inputs in, full output out. Internally: data-parallel over batch (2) x
tensor-parallel over heads (4 head-groups of 4 heads) = 8 NeuronCores.

Per core: multi-head attention on 4 heads (= 2 pairs).  Head pair m is
stacked on the partition axis (head 2m at partitions 0-63, head 2m+1 at
64-127), so the two K=64 scores matmuls of a pair run CONCURRENTLY in the
PE array as row-tiles (tile_position auto-derived from base_partition) and
one exp activation per (pair, query-block, key-tile) covers both heads at
free dim 1024.  The ScalarE exp stream (16.8M exps at 1 elem/lane/cycle)
is the throughput wall; the scalar queue carries nothing but exp, and all
other work (projections, attnV, emerge, normalize) is scheduled into the
PE/DVE slack under it.

attnV uses the ones-column trick (lhsT = [v | 1], M=65) so the softmax
denominator accumulates in PSUM partition 64 alongside the weighted sum.
The reciprocal of the denominator is broadcast across partitions with a
tiny K=1 PE matmul against a constant ones row (no DRAM round-trip).

PSUM budget (8 banks): scores 2 bufs x [128,1024]f32 (4) + two [65,512]
attnV accumulators (2) + a [128,512] pool shared by v-proj / projection
quarters / emerge / normalize-broadcast (2).  Background projections are
emitted as 1-bank "quarters" so they never stall the scores double-buffer.

The per-head "aperture chamber" (valves, rotation, tanh gate) is folded
into We host-side in float64.  Softmax max-subtraction skipped: scores are
bounded (|s| < ~7).  All matmul operands fp16.
"""

import math
from contextlib import ExitStack
import numpy as np

# ---------------------------------------------------------------- constants
P = 128          # partitions
T = 2048         # sequence length
D = 1024         # model dim
H = 16           # total heads
DH = 64          # head dim
HC = 4           # heads per core
C = HC * DH      # channels per core (256)
KT = D // P      # 8 contraction tiles over model dim
TT = T // P      # 16 tiles over sequence
MT = C // P      # 2 partition tiles (= head pairs) per core
NJ = 4           # query blocks
JW = T // NJ     # query block width (512)
NCORES = 8
SCALE = 8.0      # sqrt(dh * conv_factor), conv_factor = 1/phi^0 = 1

CFG = {
    "dt": "float16",   # matmul operand dtype (storage); accum stays f32
    "attn_ksplit": False,  # two K=64 same-bank halves per attnV MM: raises
                           # a HW error (concurrent row-tile drain to one
                           # PSUM bank) — keep False
}

LAST_EXEC_NS = None
_CACHE = {}


def _np_dt(name):
    if name == "bfloat16":
        import ml_dtypes
        return np.dtype(ml_dtypes.bfloat16)
    return np.dtype(name)


def build_nc(cfg=CFG):
    """Build + compile the single-core SPMD program."""
    import concourse.bass as bass
    import concourse.mybir as mybir
    import concourse.tile as tile
    from concourse import bacc

    dt = mybir.dt
    f32 = dt.float32
    dtx = getattr(dt, cfg["dt"])

    nc = bacc.Bacc("TRN2", target_bir_lowering=False, debug=False,
                   enable_asserts=False)

    xT = nc.dram_tensor("xt", [D, T], dtx, kind="ExternalInput").ap()
    wq = nc.dram_tensor("wq", [D, C], dtx, kind="ExternalInput").ap()
    wk = nc.dram_tensor("wk", [D, C], dtx, kind="ExternalInput").ap()
    wv = nc.dram_tensor("wv", [D, C], dtx, kind="ExternalInput").ap()
    we = nc.dram_tensor("we", [C, D], dtx, kind="ExternalInput").ap()
    out = nc.dram_tensor("out", [T, D], dtx, kind="ExternalOutput").ap()

    Exp = mybir.ActivationFunctionType.Exp

    with tile.TileContext(nc) as tc, ExitStack() as ctx:
        cp = ctx.enter_context(tc.tile_pool(name="const", bufs=1))
        psp = ctx.enter_context(tc.tile_pool(name="psum", bufs=2,
                                             space="PSUM"))
        pso = ctx.enter_context(tc.tile_pool(name="psum_o", bufs=1,
                                             space="PSUM"))
        pse = ctx.enter_context(tc.tile_pool(name="psum_ve", bufs=2,
                                             space="PSUM"))
        p_pool = ctx.enter_context(tc.tile_pool(name="pp", bufs=6))
        u_pool = ctx.enter_context(tc.tile_pool(name="usb", bufs=2))
        nrm_b = ctx.enter_context(tc.tile_pool(name="nrm_b", bufs=2))
        nrm_d = ctx.enter_context(tc.tile_pool(name="nrm_d", bufs=2,
                                               space="DRAM"))
        out_pool = ctx.enter_context(tc.tile_pool(name="oute", bufs=2))

        xT_sb = cp.tile([P, KT, T], dtx)
        wq_sb = cp.tile([P, KT, C], dtx)
        wk_sb = cp.tile([P, KT, C], dtx)
        wv_sb = cp.tile([P, KT, C], dtx)
        we_sb = cp.tile([P, MT, D], dtx)
        qT_sb = cp.tile([P, MT, T], dtx)
        kT_sb = cp.tile([P, MT, T], dtx)
        # v augmented with a ones column at index DH (softmax denominator);
        # padded to 68 for 4-byte-aligned strides. cols 65-67 never read.
        va_sb = cp.tile([P, TT, HC, 68], dtx)
        vT_sb = cp.tile([P, MT, T], dtx)
        oT_sb = cp.tile([P, MT, T], dtx)
        ones_r = cp.tile([1, DH], f32)       # lhsT of the bcast matmul
        dummy = cp.tile([1, 16], f32)
        nc.vector.memset(va_sb[:, :, :, DH:DH + 1], 1.0)
        nc.vector.memset(ones_r, 1.0)
        nc.vector.memset(dummy, 0.0)
        # warm the exp table while input DMAs run
        nc.scalar.activation(dummy, dummy, Exp)

        # ---- loads.  DMA engines fair-share bandwidth across in-flight
        # transfers, so the critical-path loads (wk/wq + xT chunks) are
        # issued strictly first; xT is chunked 6 ways across the three
        # DMA-capable queues (sync, gpsimd, scalar).
        nc.sync.dma_start(out=wk_sb, in_=wk.rearrange("(k p) c -> p k c", p=P))
        nc.gpsimd.dma_start(out=wq_sb,
                            in_=wq.rearrange("(k p) c -> p k c", p=P))
        dq = (nc.sync, nc.gpsimd, nc.scalar)
        xbounds = [0, 1, 2, 3, 4, 6, 8]
        for ch in range(6):
            a, b = xbounds[ch], xbounds[ch + 1]
            dq[ch % 3].dma_start(
                out=xT_sb[:, a:b, :],
                in_=xT[a * P:b * P, :].rearrange("(k p) t -> p k t", p=P))
        nc.scalar.dma_start(out=wv_sb,
                            in_=wv.rearrange("(k p) c -> p k c", p=P))
        nc.sync.dma_start(out=we_sb,
                            in_=we.rearrange("(m p) d -> p m d", p=P))

        def proj_kq_lead():
            """k (both halves) and q (first half) projections for pair 0,
            interleaved per k-tile so everything finishes right after the
            xT DMA.  The second k half borrows the attnV accumulator banks
            (tags o0/o1), which are idle until the first block starts."""
            psk = psp.tile([P, T // 2], f32, tag="s", name="psk")
            psq = psp.tile([P, T // 2], f32, tag="s", name="psq")
            psk2 = pso.tile([P, 512], f32, tag="o0", name="psk2")
            psk3 = pso.tile([P, 512], f32, tag="o1", name="psk3")
            for k in range(KT):
                for ps, w_sb in ((psk, wk_sb), (psq, wq_sb)):
                    for c0 in range(0, T // 2, 512):
                        nc.tensor.matmul(
                            ps[:, c0:c0 + 512],
                            lhsT=w_sb[:, k, 0:P],
                            rhs=xT_sb[:, k, c0:c0 + 512],
                            start=(k == 0), stop=(k == KT - 1),
                        )
                for qq, ps in ((2, psk2), (3, psk3)):
                    nc.tensor.matmul(
                        ps,
                        lhsT=wk_sb[:, k, 0:P],
                        rhs=xT_sb[:, k, qq * 512:(qq + 1) * 512],
                        start=(k == 0), stop=(k == KT - 1),
                    )
            for c0 in range(0, T // 2, 512):
                nc.vector.tensor_copy(kT_sb[:, 0, c0:c0 + 512],
                                      psk[:, c0:c0 + 512])
                nc.vector.tensor_copy(qT_sb[:, 0, c0:c0 + 512],
                                      psq[:, c0:c0 + 512])
            nc.vector.tensor_copy(kT_sb[:, 0, 1024:1536], psk2)
            nc.vector.tensor_copy(kT_sb[:, 0, 1536:2048], psk3)

        def proj_full_ve(w_sb, dst, m, jh):
            """1024-wide projection as two 1-bank ve tiles (lead only)."""
            for qq in (2 * jh, 2 * jh + 1):
                ps = pse.tile([P, 512], f32, tag="ve")
                for k in range(KT):
                    nc.tensor.matmul(
                        ps,
                        lhsT=w_sb[:, k, m * P:(m + 1) * P],
                        rhs=xT_sb[:, k, qq * 512:(qq + 1) * 512],
                        start=(k == 0), stop=(k == KT - 1),
                    )
                nc.vector.tensor_copy(dst[:, m, qq * 512:(qq + 1) * 512], ps)

        def proj_units(w_sb, dst, m, qq):
            """A 512-wide projection sliced into 8 single-matmul fill units
            (~320ns each) so it slots into per-kt PE slack without ever
            stalling the exp stream."""
            state = {}

            def unit():
                k = state.get("k", 0)
                if k == 0:
                    state["ps"] = pse.tile([P, 512], f32, tag="ve",
                                           name="psu")
                nc.tensor.matmul(
                    state["ps"],
                    lhsT=w_sb[:, k, m * P:(m + 1) * P],
                    rhs=xT_sb[:, k, qq * 512:(qq + 1) * 512],
                    start=(k == 0), stop=(k == KT - 1),
                )
                if k == KT - 1:
                    nc.vector.tensor_copy(
                        dst[:, m, qq * 512:(qq + 1) * 512], state["ps"])
                state["k"] = k + 1

            return [unit] * KT

        def proj_v_tile(t):
            ps = pse.tile([P, C], f32, tag="ve")
            for k in range(KT):
                nc.tensor.matmul(
                    ps,
                    lhsT=xT_sb[:, k, t * P:(t + 1) * P],
                    rhs=wv_sb[:, k, :],
                    start=(k == 0), stop=(k == KT - 1),
                )
            nc.vector.tensor_copy(
                va_sb[:, t, :, 0:DH],
                ps.rearrange("p (h d) -> p h d", h=HC))

        def normalize_bounce(prev):
            """DMA-bounce normalize of the PREVIOUS block, emitted inside
            the next block so the DRAM round-trip latency is hidden.  The
            u-copy frees the po banks within ~1us of the block start."""
            m, jq, pos = prev
            q0 = jq * JW
            for hh in range(2):
                u = u_pool.tile([DH + 1, JW], f32, tag="u")
                nc.vector.tensor_copy(u, pos[hh])
                r_dr = nrm_d.tile([1, JW], f32, tag="rd")
                eng = nc.sync if hh == 0 else nc.gpsimd
                eng.dma_start(out=r_dr, in_=u[DH:DH + 1, :])
                lbc = nrm_b.tile([DH, JW], f32, tag="lbc")
                eng.dma_start(out=lbc, in_=r_dr.to_broadcast((DH, JW)))
                rbc = nrm_b.tile([DH, JW], f32, tag="rbc")
                nc.vector.reciprocal_approx_fast(rbc, lbc)
                if hh == 0:
                    nc.vector.tensor_mul(
                        oT_sb[0:DH, m, q0:q0 + JW], u[0:DH, :], rbc)
                else:
                    st = nrm_b.tile([DH, JW], dtx, tag="st")
                    nc.vector.tensor_mul(st, u[0:DH, :], rbc)
                    eng.dma_start(out=oT_sb[DH:P, m, q0:q0 + JW], in_=st)

        def normalize_tail(prev):
            """Fast-path normalize for the last block: reciprocal row
            broadcast via a K=1 PE matmul, scalar engine assists."""
            m, jq, pos = prev
            q0 = jq * JW
            for hh in range(2):
                uv = u_pool.tile([DH, JW], f32, tag="u2")
                ud = u_pool.tile([1, JW], f32, tag="ud")
                (nc.vector.tensor_copy if hh == 0 else nc.scalar.copy)(
                    uv, pos[hh][0:DH, :])
                (nc.scalar.copy if hh == 0 else nc.vector.tensor_copy)(
                    ud, pos[hh][DH:DH + 1, :])
                rr = nrm_b.tile([1, JW], f32, tag="rr")
                nc.vector.reciprocal_approx_fast(rr, ud)
                rb = pse.tile([DH, JW], f32, tag="ve")
                nc.tensor.matmul(rb, lhsT=ones_r, rhs=rr,
                                 start=True, stop=True)
                if hh == 0:
                    nc.vector.tensor_mul(
                        oT_sb[0:DH, m, q0:q0 + JW], uv, rb)
                else:
                    st = nrm_b.tile([DH, JW], dtx, tag="st")
                    nc.vector.tensor_mul(st, uv, rb)
                    nc.gpsimd.dma_start(
                        out=oT_sb[DH:P, m, q0:q0 + JW], in_=st)

        def attention(m, jq, fills=None, pre_kt=None, norm_prev=None):
            """Head pair m, query block jq: scores (row-tiled pair), exp
            (one call for both heads), attnV with ones-column denominator.
            fills: {kt: emitter} of background PE work slotted after the
            scores of that kt.  Returns (m, jq, po-tiles) for deferred
            normalization inside the next block."""
            q0 = jq * JW
            pos = [pso.tile([DH + 1, JW], f32, tag=f"o{hh}", name=f"po{hh}")
                   for hh in range(2)]
            for kt in range(TT):
                ps = psp.tile([P, 2 * JW], f32, tag="s")
                for hh in range(2):
                    nc.tensor.matmul(
                        ps[:, hh * JW:(hh + 1) * JW],
                        lhsT=kT_sb[hh * DH:(hh + 1) * DH, m,
                                   kt * P:(kt + 1) * P],
                        rhs=qT_sb[hh * DH:(hh + 1) * DH, m, q0:q0 + JW],
                        start=True, stop=True,
                    )
                if kt == 0 and norm_prev is not None:
                    normalize_bounce(norm_prev)
                for u in (fills.get(kt, ()) if fills else ()):
                    u()
                if pre_kt is not None:
                    pre_kt(kt)
                p_t = p_pool.tile([P, 2 * JW], dtx, tag="p")
                nc.scalar.activation(p_t, ps, Exp)
                for hh in range(2):
                    if cfg["attn_ksplit"]:
                        for rg in range(2):
                            nc.tensor.matmul(
                                pos[hh],
                                lhsT=va_sb[rg * DH:(rg + 1) * DH, kt,
                                           2 * m + hh, 0:DH + 1],
                                rhs=p_t[rg * DH:(rg + 1) * DH,
                                        hh * JW:(hh + 1) * JW],
                                start=(kt == 0 and rg == 0),
                                stop=(kt == TT - 1 and rg == 1),
                            )
                    else:
                        nc.tensor.matmul(
                            pos[hh],
                            lhsT=va_sb[:, kt, 2 * m + hh, 0:DH + 1],
                            rhs=p_t[:, hh * JW:(hh + 1) * JW],
                            start=(kt == 0), stop=(kt == TT - 1),
                        )
            return (m, jq, pos)

        def emerge_units(t, tail=False):
            """out[t,:] as two ~1.1us fill units (one per 512-chunk)."""
            state = {}

            def unit():
                ci = state.get("ci", 0)
                if ci == 0:
                    state["ob"] = out_pool.tile([P, D], dtx, tag="ob",
                                                name="ob")
                ob = state["ob"]
                c0 = ci * 512
                pe = pse.tile([P, 512], f32, tag="ve", name="pe")
                for mm in range(MT):
                    nc.tensor.matmul(
                        pe,
                        lhsT=oT_sb[:, mm, t * P:(t + 1) * P],
                        rhs=we_sb[:, mm, c0:c0 + 512],
                        start=(mm == 0), stop=(mm == MT - 1),
                    )
                if tail and ci == 1:
                    nc.scalar.copy(ob[:, c0:c0 + 512], pe)
                else:
                    nc.vector.tensor_copy(ob[:, c0:c0 + 512], pe)
                if ci == 1:
                    eng = nc.sync if t % 2 == 0 else nc.gpsimd
                    eng.dma_start(out=out[t * P:(t + 1) * P, :], in_=ob)
                state["ci"] = ci + 1

            return [unit, unit]

        def spread(units, kts):
            """Distribute fill units round-robin over kt positions."""
            fills = {}
            n, s = len(units), len(kts)
            i = 0
            for idx, kt in enumerate(kts):
                take = (n * (idx + 1)) // s - (n * idx) // s
                if take:
                    fills[kt] = units[i:i + take]
                i += take
            return fills

        # ---- emission = per-engine queue order.  Lead: k/q(pair0,jh0)
        # under the xT DMA, then k(0,jh1) + v tiles 0-9.  All m=0 blocks
        # first; pair-1 projections are single-matmul fill units spread
        # through m=0 blocks; emerges spread through m=1 blocks; each
        # block's normalize is a kt=0 fill of the NEXT block.
        proj_kq_lead()
        for t in range(10):
            proj_v_tile(t)
        pv = attention(0, 0, fills={
            kt: [lambda t=kt: proj_v_tile(t)] for kt in range(10, TT)})
        pv = attention(0, 1, norm_prev=pv, fills=spread(
            proj_units(wq_sb, qT_sb, 0, 2) + proj_units(wq_sb, qT_sb, 0, 3),
            range(TT)))
        pv = attention(0, 2, norm_prev=pv, fills=spread(
            proj_units(wk_sb, kT_sb, 1, 0) + proj_units(wk_sb, kT_sb, 1, 1)
            + proj_units(wq_sb, qT_sb, 1, 0), range(TT)))
        pv = attention(0, 3, norm_prev=pv, fills=spread(
            proj_units(wk_sb, kT_sb, 1, 2) + proj_units(wk_sb, kT_sb, 1, 3)
            + proj_units(wq_sb, qT_sb, 1, 1), range(TT)))
        pv = attention(1, 0, norm_prev=pv, fills=spread(
            proj_units(wq_sb, qT_sb, 1, 2) + proj_units(wq_sb, qT_sb, 1, 3),
            range(8)))
        pv = attention(1, 1, norm_prev=pv, fills=spread(
            emerge_units(0) + emerge_units(1) + emerge_units(2)
            + emerge_units(3), range(7, 15)))
        pv = attention(1, 2, norm_prev=pv, fills=spread(
            emerge_units(4) + emerge_units(5) + emerge_units(6)
            + emerge_units(7), range(7, 15)))
        pv = attention(1, 3, norm_prev=pv, fills=spread(
            emerge_units(8) + emerge_units(9) + emerge_units(10)
            + emerge_units(11), range(4, 12)))
        normalize_tail(pv)
        for t in range(12, 16):
            for u in emerge_units(t, tail=True):
                u()

    nc.compile()
    return nc


def prep_inputs(x, Wq, Wk, Wv, We, beta, input_valve, output_valve, chi,
                cfg=CFG):
    """Host-side prep: fold chamber into We, fold 1/scale into Wq, shard."""
    x = np.asarray(x, np.float32)
    Wq = np.asarray(Wq, np.float32)
    Wk = np.asarray(Wk, np.float32)
    Wv = np.asarray(Wv, np.float32)
    We = np.asarray(We, np.float32)

    def sig(v):
        return 1.0 / (1.0 + np.exp(-np.asarray(v, np.float64)))

    b = sig(beta)
    iv = sig(input_valve)
    ov = sig(output_valve)
    g = np.tanh(np.asarray(chi, np.float64))
    ang = math.pi * b
    ca, sa = np.cos(ang), np.sin(ang)
    half = DH // 2

    We64 = We.astype(np.float64)
    WeP = np.empty((D, D), np.float64)
    for h in range(H):
        L = np.zeros((DH, DH))
        idx = np.arange(half)
        L[idx, idx] = ca[h]
        L[idx, half + idx] = -sa[h]
        L[half + idx, idx] = sa[h]
        L[half + idx, half + idx] = ca[h]
        L *= ov[h] * g[h] * iv[h]
        WeP[:, h * DH:(h + 1) * DH] = We64[:, h * DH:(h + 1) * DH] @ L

    dt_x = _np_dt(cfg["dt"])
    WqT = np.ascontiguousarray((Wq.astype(np.float64) / SCALE).T, dt_x)
    WkT = np.ascontiguousarray(Wk.T, dt_x)
    WvT = np.ascontiguousarray(Wv.T, dt_x)
    WeT = np.ascontiguousarray(WeP.T, dt_x)   # [c, dout]

    in_maps = []
    for core in range(NCORES):
        bidx, grp = divmod(core, H // HC)
        cols = slice(grp * C, (grp + 1) * C)
        in_maps.append({
            "xt": np.ascontiguousarray(x[bidx].T.astype(dt_x)),
            "wq": np.ascontiguousarray(WqT[:, cols]),
            "wk": np.ascontiguousarray(WkT[:, cols]),
            "wv": np.ascontiguousarray(WvT[:, cols]),
            "we": np.ascontiguousarray(WeT[cols, :]),
        })
    return in_maps


def kernel(**inputs):
    global LAST_EXEC_NS
    import os
    if "nc" not in _CACHE:
        _CACHE["nc"] = build_nc()
    nc = _CACHE["nc"]
    in_maps = prep_inputs(**inputs)

    from concourse.bass_utils import run_bass_kernel_spmd
    trace = bool(os.environ.get("CIRC_TRACE"))
    res = run_bass_kernel_spmd(nc, in_maps, list(range(NCORES)), trace=trace)
    LAST_EXEC_NS = res.exec_time_ns
    _CACHE["last_results"] = res

    B = 2
    outp = np.zeros((B, T, D), np.float32)
    per_batch = NCORES // B
    for core in range(NCORES):
        outp[core // per_batch] += res.results[core]["out"]
    return outp
